# revision 1
# baseline (speedup 1.0000x reference)
"""Trainium2 Bass kernel for BasicTransformerBlockST (spatial/temporal transformer block).

Sharding over 8 NeuronCores:
  Phase A (spatial self-attn): data-parallel over (b,t): core i owns the 4
  groups bt = i + 8g, so every core holds both batches.
  An 8-way on-device AllToAll reshards to (b,h,w)-parallel: core j owns rows
  (b=j//4, hw in [144*(j%4), 144*(j%4+1))), tokens r-major (token = r*16 + t).
  Phases temporal-1, cross-attn, temporal-2, FFN run on that shard.

Matmul operands bf16 (fp32 PSUM accumulation); residual stream, LN and softmax
statistics fp32. Residual stream lives in DRAM between phases.
"""

import sys

sys.path.insert(0, "/opt/trn_rl_repo")

import numpy as np
import ml_dtypes

import concourse.bass as bass
import concourse.bacc as bacc
import concourse.mybir as mybir
import concourse.tile as tile
from concourse.masks import make_identity

F32 = mybir.dt.float32
BF16 = mybir.dt.bfloat16
AF = mybir.ActivationFunctionType
ALU = mybir.AluOpType

B, C, T, H, W = 2, 640, 16, 24, 24
HEADS, DH = 8, 80
CTXD = 1024
MAXREL = 16
NREL = 2 * MAXREL + 1          # 33
FFI = 4 * C                    # 2560
INNER = HEADS * DH             # 640
SCALE = DH ** -0.5
EPS = 1e-5

NCORES = 8
NG = 4                         # spatial groups per core
SEQ = H * W                    # 576
NR = (B * H * W) // NCORES     # 144 rows per core
TOK = NR * T                   # 2304 tokens per core
NWIN = TOK // 128              # 18
GW = 5                         # windows per padded spatial group
CHUNKS = C // 128              # 5
CTXCH = CTXD // 128            # 8
HALFW = NWIN // 2              # 9 windows per temporal half
HR = NR // 2                   # 72 rows per half


def nsplits(n, cap=512):
    out, o = [], 0
    while o < n:
        out.append((o, min(cap, n - o)))
        o += min(cap, n - o)
    return out


def build_program(debug=False):
    nc = bacc.Bacc(None, target_bir_lowering=False)

    xs_in = nc.dram_tensor("xs_in", [NG, SEQ, C], F32, kind="ExternalInput")
    ctxT_in = nc.dram_tensor("ctxT", [CTXD, 77], BF16, kind="ExternalInput")

    def win(name, shape, dt=BF16):
        return nc.dram_tensor(name, shape, dt, kind="ExternalInput")

    wts, biases = {}, {}
    for p in ("a1", "a2", "t1", "t2"):
        cin = CTXD if p == "a2" else C
        wts[f"{p}_wq"] = win(f"{p}_wq", [C, INNER])
        wts[f"{p}_wk"] = win(f"{p}_wk", [cin, INNER])
        wts[f"{p}_wv"] = win(f"{p}_wv", [cin, INNER])
        wts[f"{p}_wo"] = win(f"{p}_wo", [DH, HEADS, C])
        biases[f"{p}_bq"] = win(f"{p}_bq", [INNER], F32)
        biases[f"{p}_bk"] = win(f"{p}_bk", [INNER], F32)
        biases[f"{p}_bv"] = win(f"{p}_bv", [INNER], F32)
        biases[f"{p}_bo"] = win(f"{p}_bo", [C], F32)
    for p in ("t1", "t2"):
        wts[f"{p}_rkT"] = win(f"{p}_rkT", [DH, NREL])
        wts[f"{p}_rvs"] = win(f"{p}_rvs", [16, T, DH])  # rvs[j,t,d]=rv[j-t+16,d]
    wts["ff_w1"] = win("ff_w1", [C, 2 * FFI])
    wts["ff_w2"] = win("ff_w2", [FFI, C])
    biases["ff_b1"] = win("ff_b1", [2 * FFI], F32)
    biases["ff_b2"] = win("ff_b2", [C], F32)
    bd_mask = win("bd_mask", [128, 128], F32)

    out_final = nc.dram_tensor("out", [NR, T, C], F32, kind="ExternalOutput")
    dbg = {}
    if debug:
        dbg["a"] = nc.dram_tensor("dbg_a", [NG, SEQ, C], F32, kind="ExternalOutput")
        for nm in ("t1", "x2", "t2"):
            dbg[nm] = nc.dram_tensor(f"dbg_{nm}", [NR, T, C], F32,
                                     kind="ExternalOutput")

    a2a_in = nc.dram_tensor("a2a_in", [NCORES, NR, 2, C], F32)
    a2a_out = nc.dram_tensor("a2a_out", [NCORES, NR, 2, C], F32)
    x_dram = nc.dram_tensor("x_dram", [TOK, C], F32)
    sim2_dram = nc.dram_tensor("sim2_dram", [TOK, HEADS, 16], BF16)
    groups = [[0, 1, 2, 3, 4, 5, 6, 7]]

    from contextlib import ExitStack

    with tile.TileContext(nc) as tc, ExitStack() as top:
        const = top.enter_context(tc.tile_pool(name="const", bufs=1))
        ident = const.tile([128, 128], F32)
        make_identity(nc, ident)
        identb = const.tile([128, 128], BF16)
        make_identity(nc, identb)
        eps_t = const.tile([128, 1], F32)
        nc.vector.memset(eps_t[:], EPS)
        mask = const.tile([128, 128], F32)
        nc.sync.dma_start(out=mask[:], in_=bd_mask[:, :])
        small = top.enter_context(tc.tile_pool(name="small", bufs=4))
        zscr = top.enter_context(tc.tile_pool(name="zscr", bufs=2))

        def bcast_tile(wp, name, n=C):
            t = wp.tile([128, n], F32, tag=f"bc_{name}")
            src = biases[name][:]
            bc = bass.AP(tensor=src.tensor, offset=src.offset,
                         ap=[[0, 128], [1, n]])
            nc.gpsimd.dma_start(out=t[:], in_=bc)
            return t

        # ---------------- shared helpers ----------------
        def ln_to_fm(psp, x_ap, zT_tile, nw):
            """LN over channels + transpose: x [128,nw,640] f32 ->
            zT [128,CHUNKS,nw*128] bf16 feature-major (normalized, no g/b)."""
            for w in range(nw):
                x = x_ap[:, w, :]
                st = small.tile([128, CHUNKS, 6], F32, tag="bnst")
                for s in range(CHUNKS):
                    nc.vector.bn_stats(out=st[:, s, :],
                                       in_=x[:, 128 * s:128 * (s + 1)])
                mv = small.tile([128, 2], F32, tag="bnmv")
                nc.vector.bn_aggr(out=mv[:], in_=st[:])
                rstd = small.tile([128, 1], F32, tag="rstd")
                nc.scalar.activation(out=rstd[:], in_=mv[:, 1:2], func=AF.Sqrt,
                                     bias=eps_t[:], scale=1.0)
                nc.vector.reciprocal(out=rstd[:], in_=rstd[:])
                zs = zscr.tile([128, C], F32, tag="zs")
                nc.vector.tensor_scalar(
                    out=zs[:], in0=x, scalar1=mv[:, 0:1], scalar2=rstd[:],
                    op0=ALU.subtract, op1=ALU.mult)
                for c in range(CHUNKS):
                    pt = psp.tile([128, 128], F32, tag="ps")
                    nc.tensor.transpose(pt[:], zs[:, 128 * c:128 * (c + 1)],
                                        ident[:])
                    nc.vector.tensor_scalar_mul(
                        out=zT_tile[:, c, 128 * w:128 * (w + 1)], in0=pt[:],
                        scalar1=1.0)

        def proj_heads(psp, zT, w_sb, out_tile, ntok, bias=None,
                       cin_chunks=CHUNKS):
            """per-head feature-major projection: out [80, HEADS, ntok] bf16."""
            for h in range(HEADS):
                for (o, n) in nsplits(ntok):
                    pt = psp.tile([128, 512], F32, tag="ps")
                    for ci in range(cin_chunks):
                        nc.tensor.matmul(pt[:DH, :n],
                                         w_sb[:, ci, DH * h:DH * (h + 1)],
                                         zT[:, ci, o:o + n],
                                         start=(ci == 0),
                                         stop=(ci == cin_chunks - 1))
                    if bias is not None:
                        nc.vector.tensor_scalar_add(out=out_tile[:, h, o:o + n],
                                                    in0=pt[:DH, :n],
                                                    scalar1=bias[:, h:h + 1])
                    else:
                        nc.vector.tensor_scalar_mul(out=out_tile[:, h, o:o + n],
                                                    in0=pt[:DH, :n], scalar1=1.0)

        def proj_tm(psp, zT, w_sb, out_tile, tok_chunks, badd=None):
            """token-major: out[tok, 640]; lhsT = zT[:,ci,toks], rhs = W."""
            for (w, p, toff) in tok_chunks:
                for (o, n) in nsplits(C):
                    pt = psp.tile([128, 512], F32, tag="ps")
                    for ci in range(CHUNKS):
                        nc.tensor.matmul(pt[:p, :n],
                                         zT[:, ci, toff:toff + p],
                                         w_sb[:, ci, o:o + n],
                                         start=(ci == 0), stop=(ci == CHUNKS - 1))
                    if badd is not None:
                        nc.vector.tensor_add(out=out_tile[:p, w, o:o + n],
                                             in0=pt[:p, :n],
                                             in1=badd[:p, o:o + n])
                    else:
                        nc.scalar.copy(out=out_tile[:p, w, o:o + n],
                                       in_=pt[:p, :n])

        def wo_residual(psp, oT, wo, w, resid_ap, bo):
            """by-head wo projection + bias + residual-add into resid_ap."""
            mp = resid_ap.shape[0]
            for (o, n) in nsplits(C):
                pw = psp.tile([128, 512], F32, tag="ps")
                for h in range(HEADS):
                    nc.tensor.matmul(pw[:mp, :n],
                                     oT[:, h, 128 * w:128 * w + mp],
                                     wo[:, h, o:o + n],
                                     start=(h == 0), stop=(h == HEADS - 1))
                nc.vector.tensor_add(out=resid_ap[:, o:o + n], in0=pw[:mp, :n],
                                     in1=resid_ap[:, o:o + n])
            nc.vector.tensor_add(out=resid_ap[:], in0=resid_ap[:], in1=bo[:mp, :])

        def load_w_cin(wp, name, cin):
            t = wp.tile([128, cin // 128, wts[name].shape[-1]], BF16, tag=name[3:])
            nc.sync.dma_start(out=t[:],
                              in_=wts[name][:].rearrange("(a p) n -> p a n", p=128))
            return t

        def load_wo(wp, name):
            t = wp.tile([DH, HEADS, C], BF16, tag="wo")
            nc.sync.dma_start(out=t[:], in_=wts[name][:])
            return t

        def load_bias_h(wp, name):
            t = wp.tile([DH, HEADS], F32, tag=name[3:] + "b")
            nc.sync.dma_start(out=t[:],
                              in_=biases[name][:].rearrange("(h p) -> p h", p=DH))
            return t

        # =====================================================================
        # PHASE A: spatial self-attention, per (b,t) group
        # =====================================================================
        with ExitStack() as ph:
            wp = ph.enter_context(tc.tile_pool(name="wpA", bufs=1))
            zp = ph.enter_context(tc.tile_pool(name="zpA", bufs=1))
            qp = ph.enter_context(tc.tile_pool(name="qpA", bufs=2))
            ap_ = ph.enter_context(tc.tile_pool(name="apA", bufs=1))
            psp = ph.enter_context(tc.tile_pool(name="psA", bufs=8, space="PSUM"))

            wq = load_w_cin(wp, "a1_wq", C)
            wk = load_w_cin(wp, "a1_wk", C)
            wv = load_w_cin(wp, "a1_wv", C)
            wo = load_wo(wp, "a1_wo")
            bq = load_bias_h(wp, "a1_bq")
            bk = load_bias_h(wp, "a1_bk")
            bv_b = bcast_tile(wp, "a1_bv")
            bo_b = bcast_tile(wp, "a1_bo")

            tok_chunks = [(w, 128 if w < 4 else 64, 128 * w) for w in range(GW)]

            for g in range(NG):
                xg = zp.tile([128, GW, C], F32, tag="xa")
                nc.sync.dma_start(out=xg[:, 0:4, :],
                                  in_=xs_in[g, 0:512, :].rearrange(
                                      "(a p) c -> p a c", p=128))
                nc.sync.dma_start(out=xg[:64, 4, :], in_=xs_in[g, 512:576, :])
                nc.vector.memset(xg[64:128, 4, :], 0.0)

                zT = zp.tile([128, CHUNKS, GW * 128], BF16, tag="zTa")
                ln_to_fm(psp, xg, zT, GW)

                qT = qp.tile([DH, HEADS, SEQ], BF16, tag="qa")
                kT = qp.tile([DH, HEADS, SEQ], BF16, tag="ka")
                proj_heads(psp, zT[:, :, 0:SEQ], wq, qT, SEQ, bias=bq)
                proj_heads(psp, zT[:, :, 0:SEQ], wk, kT, SEQ, bias=bk)
                v = qp.tile([128, GW, C], BF16, tag="va")
                proj_tm(psp, zT, wv, v, tok_chunks, badd=bv_b)

                oT = ap_.tile([DH, HEADS, SEQ], BF16, tag="oa")
                for h in range(HEADS):
                    a_sb = ap_.tile([128, GW, SEQ], BF16, tag="aa")
                    for (mw, mp, moff) in tok_chunks:
                        zsum = small.tile([128, 2], F32, tag="zs2")
                        ex = ap_.tile([128, SEQ], F32, tag="ex")
                        for ki, (o, n) in enumerate(nsplits(SEQ)):
                            ps = psp.tile([128, 512], F32, tag="ps")
                            nc.tensor.matmul(ps[:mp, :n],
                                             qT[:, h, moff:moff + mp],
                                             kT[:, h, o:o + n],
                                             start=True, stop=True)
                            nc.scalar.activation(
                                out=ex[:mp, o:o + n], in_=ps[:mp, :n],
                                func=AF.Exp, scale=SCALE,
                                accum_out=zsum[:mp, ki:ki + 1])
                        ztot = small.tile([128, 1], F32, tag="zt")
                        nc.vector.tensor_add(out=ztot[:mp, :], in0=zsum[:mp, 0:1],
                                             in1=zsum[:mp, 1:2])
                        nc.vector.reciprocal(out=ztot[:mp, :], in_=ztot[:mp, :])
                        nc.vector.tensor_scalar_mul(out=a_sb[:mp, mw, :],
                                                    in0=ex[:mp, :],
                                                    scalar1=ztot[:mp, :])
                    # AV: o^T[d, q] = sum_k v[k, d] a[q, k]; query chunks
                    # paired so each AV matmul streams N=256.
                    pairs = [(tok_chunks[0], tok_chunks[1]),
                             (tok_chunks[2], tok_chunks[3]),
                             (tok_chunks[4], None)]
                    for (c0, c1) in pairs:
                        np_ = c0[1] + (c1[1] if c1 else 0)
                        moff = c0[2]
                        po = psp.tile([DH, 256], F32, tag="ps")
                        for ik, (kw, kp, koff) in enumerate(tok_chunks):
                            aT = ap_.tile([128, 256], BF16, tag="aT")
                            for sub, cc in enumerate((c0, c1)):
                                if cc is None:
                                    continue
                                (mw, mp, mo) = cc
                                pa = psp.tile([128, 128], BF16, tag="ps")
                                nc.tensor.transpose(pa[:kp, :mp],
                                                    a_sb[:mp, mw, koff:koff + kp],
                                                    identb[:mp, :mp])
                                nc.scalar.copy(out=aT[:kp, 128 * sub:128 * sub + mp],
                                               in_=pa[:kp, :mp])
                            nc.tensor.matmul(po[:, :np_],
                                             v[:kp, kw, DH * h:DH * (h + 1)],
                                             aT[:kp, :np_] if np_ == 256 else
                                             aT[:kp, :np_],
                                             start=(ik == 0), stop=(ik == GW - 1))
                        nc.scalar.copy(out=oT[:, h, moff:moff + np_],
                                       in_=po[:, :np_])

                for (mw, mp, moff) in tok_chunks:
                    xn = zp.tile([128, C], F32, tag="xan")
                    nc.scalar.copy(out=xn[:mp, :], in_=xg[:mp, mw, :])
                    wo_residual(psp, oT, wo, mw, xn[:mp, :], bo_b)
                    q0, q1 = moff // NR, (moff + mp - 1) // NR
                    for q in range(q0, q1 + 1):
                        lo, hi = max(moff, NR * q), min(moff + mp, NR * (q + 1))
                        nc.sync.dma_start(
                            out=a2a_in[4 * (g // 2) + q, lo - NR * q:hi - NR * q,
                                       g % 2, :],
                            in_=xn[lo - moff:hi - moff, :])
                    if debug:
                        nc.sync.dma_start(out=dbg["a"][g, moff:moff + mp, :],
                                          in_=xn[:mp, :])

        # =====================================================================
        # AllToAll reshard
        # =====================================================================
        nc.gpsimd.collective_compute("AllToAll", ALU.bypass, replica_groups=groups,
                                     ins=[a2a_in[:]], outs=[a2a_out[:]])

        def load_x_window(dst_ap, wg, first):
            if first:
                base = a2a_out[:]
                src = bass.AP(tensor=base.tensor,
                              offset=base.offset + 8 * wg * 2 * C,
                              ap=[[2 * C, 8], [C, 2], [NR * 2 * C, 8], [1, C]])
            else:
                src = x_dram[128 * wg:128 * (wg + 1), :]
            nc.sync.dma_start(out=dst_ap, in_=src)

        # =====================================================================
        # Temporal attention (t1 / t2)
        # =====================================================================
        def temporal(prefix, dbg_key, first):
            with ExitStack() as ph:
                wp = ph.enter_context(tc.tile_pool(name="wpT", bufs=1))
                zp = ph.enter_context(tc.tile_pool(name="zpT", bufs=1))
                qp = ph.enter_context(tc.tile_pool(name="qpT", bufs=1))
                ap_ = ph.enter_context(tc.tile_pool(name="apT", bufs=2))
                op_ = ph.enter_context(tc.tile_pool(name="opT", bufs=1))
                psp = ph.enter_context(tc.tile_pool(name="psT", bufs=8,
                                                    space="PSUM"))

                wq = load_w_cin(wp, f"{prefix}_wq", C)
                wk = load_w_cin(wp, f"{prefix}_wk", C)
                wv = load_w_cin(wp, f"{prefix}_wv", C)
                wo = load_wo(wp, f"{prefix}_wo")
                bq = load_bias_h(wp, f"{prefix}_bq")
                bk = load_bias_h(wp, f"{prefix}_bk")
                bv_b = bcast_tile(wp, f"{prefix}_bv")
                bo_b = bcast_tile(wp, f"{prefix}_bo")
                rkT = wp.tile([DH, NREL], BF16, tag="rkT")
                nc.sync.dma_start(out=rkT[:], in_=wts[f"{prefix}_rkT"][:])
                rvs = wp.tile([16, T, DH], BF16, tag="rvs")
                nc.sync.dma_start(out=rvs[:], in_=wts[f"{prefix}_rvs"][:])

                for half in range(2):
                    wlo = half * HALFW
                    ntok = 128 * HALFW  # 1152
                    xw = zp.tile([128, HALFW, C], F32, tag="xw")
                    for w in range(HALFW):
                        load_x_window(xw[:, w, :], wlo + w, first)
                    zT = zp.tile([128, CHUNKS, ntok], BF16, tag="zTt")
                    ln_to_fm(psp, xw, zT, HALFW)

                    qT = qp.tile([DH, HEADS, ntok], BF16, tag="qt")
                    kT = qp.tile([DH, HEADS, ntok], BF16, tag="kt")
                    proj_heads(psp, zT, wq, qT, ntok, bias=bq)
                    proj_heads(psp, zT, wk, kT, ntok, bias=bk)
                    v = qp.tile([128, HALFW, C], BF16, tag="vt")
                    proj_tm(psp, zT, wv, v,
                            [(w, 128, 128 * w) for w in range(HALFW)],
                            badd=bv_b)

                    # rel-pos scores P^T = rk . q^T; shear-transpose into
                    # sim2 token layout, bounce via DRAM.
                    s2byT = ap_.tile([HR, T, HEADS, 16], BF16, tag="s2byT")
                    for h in range(HEADS):
                        pSB = ap_.tile([NREL, ntok], BF16, tag="pSB")
                        for (o, n) in nsplits(ntok):
                            pp = psp.tile([NREL, 512], F32, tag="ps")
                            nc.tensor.matmul(pp[:, :n], rkT[:, :],
                                             qT[:, h, o:o + n],
                                             start=True, stop=True)
                            nc.scalar.copy(out=pSB[:, o:o + n], in_=pp[:, :n])
                        for t in range(T):
                            src = bass.AP(tensor=pSB.tensor,
                                          offset=pSB[:, :].offset + t,
                                          ap=[list(pSB[:, :].ap[0]), [16, HR]])
                            pt = psp.tile([HR, NREL], BF16, tag="ps")
                            nc.tensor.transpose(pt[:], src, identb[:NREL, :NREL])
                            nc.scalar.copy(
                                out=s2byT[:, t, h, :],
                                in_=pt[:, MAXREL - t:2 * MAXREL - t])
                    dst = sim2_dram[:].rearrange("(r t) h j -> r t h j", t=T)
                    nc.sync.dma_start(out=dst[HR * half:HR * half + HR],
                                      in_=s2byT[:])

                    # attention windows
                    oT = op_.tile([DH, HEADS, ntok], BF16, tag="ot")
                    aDT = op_.tile([16, HEADS, ntok], BF16, tag="aDT")
                    for w in range(HALFW):
                        wg = wlo + w
                        s2w = ap_.tile([128, HEADS, 16], BF16, tag="s2w")
                        nc.sync.dma_start(
                            out=s2w[:],
                            in_=sim2_dram[128 * wg:128 * (wg + 1), :, :])
                        aG = ap_.tile([128, HEADS, 128], BF16, tag="aG")
                        for h in range(HEADS):
                            ps = psp.tile([128, 128], F32, tag="ps")
                            nc.tensor.matmul(ps[:],
                                             qT[:, h, 128 * w:128 * (w + 1)],
                                             kT[:, h, 128 * w:128 * (w + 1)],
                                             start=True, stop=True)
                            s2rep = bass.AP(
                                tensor=s2w.tensor,
                                offset=s2w[:, h, :].offset,
                                ap=[list(s2w[:, :, :].ap[0]), [0, 8], [1, 16]])
                            tmp = ap_.tile([128, 128], F32, tag="tmpst")
                            nc.vector.scalar_tensor_tensor(
                                out=tmp[:], in0=ps[:], scalar=1.0, in1=s2rep,
                                op0=ALU.mult, op1=ALU.add)
                            exv = ap_.tile([128, 128], F32, tag="exv")
                            nc.scalar.activation(out=exv[:], in_=tmp[:],
                                                 func=AF.Exp, scale=SCALE)
                            zsum = small.tile([128, 1], F32, tag="zsT")
                            nc.vector.scalar_tensor_tensor(
                                out=aG[:, h, :], in0=exv[:], scalar=1.0,
                                in1=mask[:], op0=ALU.mult, op1=ALU.mult,
                                accum_out=zsum[:])
                            nc.vector.reciprocal(out=zsum[:], in_=zsum[:])
                            nc.vector.tensor_scalar_mul(out=aG[:, h, :],
                                                        in0=aG[:, h, :],
                                                        scalar1=zsum[:])
                        # within-row diag blocks: off-diag of aG is zero, so
                        # aD[p,h,j] = sum_g' aG[p,h,16g'+j]
                        aD = ap_.tile([128, HEADS, 16], F32, tag="aD")
                        agv = bass.AP(
                            tensor=aG.tensor, offset=aG[:, :, :].offset,
                            ap=[list(aG[:, :, :].ap[0]), [128, HEADS],
                                [1, 16], [16, 8]])
                        nc.vector.tensor_reduce(
                            out=aD[:], in_=agv, axis=mybir.AxisListType.X,
                            op=ALU.add)
                        for h in range(HEADS):
                            paT = psp.tile([128, 128], BF16, tag="ps")
                            nc.tensor.transpose(paT[:], aG[:, h, :], identb[:])
                            aTs = ap_.tile([128, 128], BF16, tag="aTs")
                            nc.scalar.copy(out=aTs[:], in_=paT[:])
                            po = psp.tile([DH, 128], F32, tag="ps")
                            nc.tensor.matmul(po[:], v[:, w, DH * h:DH * (h + 1)],
                                             aTs[:], start=True, stop=True)
                            nc.scalar.copy(out=oT[:, h, 128 * w:128 * (w + 1)],
                                           in_=po[:])
                            pd = psp.tile([16, 128], F32, tag="ps")
                            nc.tensor.transpose(pd[:], aD[:, h, :], ident[:])
                            nc.scalar.copy(out=aDT[:, h, 128 * w:128 * (w + 1)],
                                           in_=pd[:])
                    # o2: per (t, h): lhsT = pre-shifted rv slice, rhs = aDT cols
                    for t in range(T):
                        for h in range(HEADS):
                            rhs = bass.AP(tensor=aDT.tensor,
                                          offset=aDT[:, h, :].offset + t,
                                          ap=[list(aDT[:, :, :].ap[0]), [16, HR]])
                            p2 = psp.tile([DH, HR], F32, tag="ps")
                            nc.tensor.matmul(p2[:], rvs[:, t, :], rhs,
                                             start=True, stop=True)
                            dstp = bass.AP(tensor=oT.tensor,
                                           offset=oT[:, h, :].offset + t,
                                           ap=[list(oT[:, :, :].ap[0]), [16, HR]])
                            nc.vector.tensor_add(out=dstp, in0=p2[:], in1=dstp)
                    # wo + residual, store to x_dram
                    for w in range(HALFW):
                        wg = wlo + w
                        wo_residual(psp, oT, wo, w, xw[:, w, :], bo_b)
                        nc.sync.dma_start(out=x_dram[128 * wg:128 * (wg + 1), :],
                                          in_=xw[:, w, :])
                        if debug:
                            nc.sync.dma_start(
                                out=dbg[dbg_key][:].rearrange(
                                    "r t c -> (r t) c")[128 * wg:128 * (wg + 1), :],
                                in_=xw[:, w, :])

        temporal("t1", "t1", first=True)

        # =====================================================================
        # Cross-attention (processed in halves)
        # =====================================================================
        with ExitStack() as ph:
            wp = ph.enter_context(tc.tile_pool(name="wpX", bufs=1))
            zp = ph.enter_context(tc.tile_pool(name="zpX", bufs=2))
            qp = ph.enter_context(tc.tile_pool(name="qpX", bufs=1))
            ap_ = ph.enter_context(tc.tile_pool(name="apX", bufs=2))
            op_ = ph.enter_context(tc.tile_pool(name="opX", bufs=2))
            psp = ph.enter_context(tc.tile_pool(name="psX", bufs=8, space="PSUM"))

            wq = load_w_cin(wp, "a2_wq", C)
            wkc = load_w_cin(wp, "a2_wk", CTXD)
            wvc = load_w_cin(wp, "a2_wv", CTXD)
            wo = load_wo(wp, "a2_wo")
            bq = load_bias_h(wp, "a2_bq")
            bo_b = bcast_tile(wp, "a2_bo")

            ctx_sb = wp.tile([128, CTXCH, 77], BF16, tag="ctx")
            nc.sync.dma_start(out=ctx_sb[:],
                              in_=ctxT_in[:].rearrange("(a p) m -> p a m", p=128))
            kctxT = wp.tile([DH, HEADS, 77], BF16, tag="kctx")
            for h in range(HEADS):
                pt = psp.tile([128, 77], F32, tag="ps")
                for ci in range(CTXCH):
                    nc.tensor.matmul(pt[:DH, :], wkc[:, ci, DH * h:DH * (h + 1)],
                                     ctx_sb[:, ci, :],
                                     start=(ci == 0), stop=(ci == CTXCH - 1))
                nc.scalar.copy(out=kctxT[:, h, :], in_=pt[:DH, :])
            vctx = wp.tile([77, C], BF16, tag="vctx")
            for (o, n) in nsplits(C):
                pt = psp.tile([77, 512], F32, tag="ps")
                for ci in range(CTXCH):
                    nc.tensor.matmul(pt[:, :n], ctx_sb[:, ci, :],
                                     wvc[:, ci, o:o + n],
                                     start=(ci == 0), stop=(ci == CTXCH - 1))
                nc.scalar.copy(out=vctx[:, o:o + n], in_=pt[:, :n])

            for half in range(2):
                wlo = half * HALFW
                ntok = 128 * HALFW
                xw = zp.tile([128, HALFW, C], F32, tag="xwx")
                for w in range(HALFW):
                    load_x_window(xw[:, w, :], wlo + w, False)
                zT = zp.tile([128, CHUNKS, ntok], BF16, tag="zTx")
                ln_to_fm(psp, xw, zT, HALFW)
                qT = qp.tile([DH, HEADS, ntok], BF16, tag="qx")
                proj_heads(psp, zT, wq, qT, ntok, bias=bq)

                oT = op_.tile([DH, HEADS, ntok], BF16, tag="ox")
                for w in range(HALFW):
                    for h in range(HEADS):
                        ps = psp.tile([128, 77], F32, tag="ps")
                        nc.tensor.matmul(ps[:], qT[:, h, 128 * w:128 * (w + 1)],
                                         kctxT[:, h, :], start=True, stop=True)
                        ex = ap_.tile([128, 77], F32, tag="exx")
                        zsum = small.tile([128, 1], F32, tag="zsX")
                        nc.scalar.activation(out=ex[:], in_=ps[:], func=AF.Exp,
                                             scale=SCALE, accum_out=zsum[:])
                        nc.vector.reciprocal(out=zsum[:], in_=zsum[:])
                        ab = ap_.tile([128, 77], BF16, tag="abx")
                        nc.vector.tensor_scalar_mul(out=ab[:], in0=ex[:],
                                                    scalar1=zsum[:])
                        paT = psp.tile([77, 128], BF16, tag="ps")
                        nc.tensor.transpose(paT[:], ab[:], identb[:])
                        aT = ap_.tile([77, 128], BF16, tag="aTx")
                        nc.scalar.copy(out=aT[:], in_=paT[:])
                        po = psp.tile([DH, 128], F32, tag="ps")
                        nc.tensor.matmul(po[:], vctx[:, DH * h:DH * (h + 1)],
                                         aT[:], start=True, stop=True)
                        nc.scalar.copy(out=oT[:, h, 128 * w:128 * (w + 1)],
                                       in_=po[:])
                for w in range(HALFW):
                    wg = wlo + w
                    wo_residual(psp, oT, wo, w, xw[:, w, :], bo_b)
                    nc.sync.dma_start(out=x_dram[128 * wg:128 * (wg + 1), :],
                                      in_=xw[:, w, :])
                    if debug:
                        nc.sync.dma_start(
                            out=dbg["x2"][:].rearrange(
                                "r t c -> (r t) c")[128 * wg:128 * (wg + 1), :],
                            in_=xw[:, w, :])

        temporal("t2", "t2", first=False)

        # =====================================================================
        # GEGLU FFN (slices of 3 windows)
        # =====================================================================
        with ExitStack() as ph:
            wp = ph.enter_context(tc.tile_pool(name="wpF", bufs=1))
            zp = ph.enter_context(tc.tile_pool(name="zpF", bufs=1))
            hp = ph.enter_context(tc.tile_pool(name="hpF", bufs=2))
            psp = ph.enter_context(tc.tile_pool(name="psF", bufs=8, space="PSUM"))

            w1 = wp.tile([128, CHUNKS, 2 * FFI], BF16, tag="w1")
            nc.sync.dma_start(out=w1[:],
                              in_=wts["ff_w1"][:].rearrange("(a p) n -> p a n",
                                                            p=128))
            w2 = wp.tile([128, FFI // 128, C], BF16, tag="w2")
            nc.sync.dma_start(out=w2[:],
                              in_=wts["ff_w2"][:].rearrange("(a p) n -> p a n",
                                                            p=128))
            b1 = wp.tile([128, 2 * FFI // 128], F32, tag="b1")
            nc.sync.dma_start(out=b1[:],
                              in_=biases["ff_b1"][:].rearrange("(a p) -> p a",
                                                               p=128))
            fb2 = bcast_tile(wp, "ff_b2")

            SW = 3
            NG2 = FFI // 128  # 20
            for s in range(NWIN // SW):
                wlo = s * SW
                ntok = 128 * SW
                xw = zp.tile([128, SW, C], F32, tag="xwf")
                for w in range(SW):
                    load_x_window(xw[:, w, :], wlo + w, False)
                zT = zp.tile([128, CHUNKS, ntok], BF16, tag="zTf")
                ln_to_fm(psp, xw, zT, SW)
                hT = hp.tile([128, 2 * NG2, ntok], BF16, tag="hT")
                for co in range(2 * NG2):
                    pt = psp.tile([128, ntok], F32, tag="ps")
                    for ci in range(CHUNKS):
                        nc.tensor.matmul(pt[:], w1[:, ci, 128 * co:128 * (co + 1)],
                                         zT[:, ci, :],
                                         start=(ci == 0), stop=(ci == CHUNKS - 1))
                    nc.scalar.activation(out=hT[:, co, :], in_=pt[:],
                                         func=AF.Identity,
                                         bias=b1[:, co:co + 1], scale=1.0)
                uT = hp.tile([128, NG2, ntok], BF16, tag="uT")
                for co in range(NG2):
                    gl = hp.tile([128, ntok], BF16, tag="gelu")
                    nc.scalar.activation(out=gl[:], in_=hT[:, NG2 + co, :],
                                         func=AF.Gelu)
                    nc.vector.tensor_mul(out=uT[:, co, :], in0=hT[:, co, :],
                                         in1=gl[:])
                for w in range(SW):
                    wg = wlo + w
                    for (o, n) in nsplits(C):
                        pw = psp.tile([128, 512], F32, tag="ps")
                        for ci in range(NG2):
                            nc.tensor.matmul(pw[:, :n],
                                             uT[:, ci, 128 * w:128 * (w + 1)],
                                             w2[:, ci, o:o + n],
                                             start=(ci == 0), stop=(ci == NG2 - 1))
                        nc.vector.tensor_add(out=xw[:, w, o:o + n], in0=pw[:, :n],
                                             in1=xw[:, w, o:o + n])
                    nc.vector.tensor_add(out=xw[:, w, :], in0=xw[:, w, :],
                                         in1=fb2[:])
                    nc.sync.dma_start(
                        out=out_final[:].rearrange(
                            "r t c -> (r t) c")[128 * wg:128 * (wg + 1), :],
                        in_=xw[:, w, :])

    if not nc.is_finalized():
        nc.finalize()
    return nc


# ----------------------------------------------------------------------------
# host side
# ----------------------------------------------------------------------------

def _bf(a):
    return np.asarray(a, dtype=ml_dtypes.bfloat16)


def prepare_inputs(inputs):
    f = {k: np.asarray(v, dtype=np.float32) for k, v in inputs.items()}
    shared = {}

    def fold(g, b, wname):
        wf = f[wname]
        return f[g][:, None] * wf, f[b] @ wf

    for p, gk, bk_ in (("a1", "g1", "b1"), ("t1", "g4", "b4"),
                       ("t2", "g5", "b5")):
        for kind in ("wq", "wk", "wv"):
            wf, bias = fold(gk, bk_, f"{p}_{kind}")
            shared[f"{p}_{kind}"] = _bf(wf)
            shared[f"{p}_b{kind[1]}"] = bias.astype(np.float32)
    wf, bias = fold("g2", "b2", "a2_wq")
    shared["a2_wq"] = _bf(wf)
    shared["a2_bq"] = bias.astype(np.float32)
    shared["a2_wk"] = _bf(f["a2_wk"])
    shared["a2_wv"] = _bf(f["a2_wv"])
    shared["a2_bk"] = np.zeros(INNER, np.float32)
    shared["a2_bv"] = np.zeros(INNER, np.float32)
    for p in ("a1", "a2", "t1", "t2"):
        shared[f"{p}_wo"] = _bf(
            f[f"{p}_wo"].reshape(HEADS, DH, C).transpose(1, 0, 2))
        shared[f"{p}_bo"] = f[f"{p}_bo"]
    for p in ("t1", "t2"):
        shared[f"{p}_rkT"] = _bf(f[f"{p}_rk"].T)
        rv = f[f"{p}_rv"]
        rvs = np.zeros((16, T, DH), np.float32)
        for t in range(T):
            for j in range(16):
                rvs[j, t] = rv[j - t + MAXREL]
        shared[f"{p}_rvs"] = _bf(rvs)
    w1f, b1f = fold("g3", "b3", "ff_w1")
    shared["ff_w1"] = _bf(w1f)
    shared["ff_b1"] = (b1f + f["ff_b1"]).astype(np.float32)
    shared["ff_w2"] = _bf(f["ff_w2"])
    shared["ff_b2"] = f["ff_b2"]
    m = np.zeros((128, 128), np.float32)
    for g in range(8):
        m[16 * g:16 * (g + 1), 16 * g:16 * (g + 1)] = 1.0
    shared["bd_mask"] = m

    x = f["x"]
    ctx = f["context"]
    in_maps = []
    for core in range(NCORES):
        im = dict(shared)
        xs = np.empty((NG, SEQ, C), np.float32)
        for g in range(NG):
            bt = core + 8 * g
            b, t = bt // T, bt % T
            xs[g] = x[b, :, t].reshape(C, SEQ).T
        im["xs_in"] = xs
        im["ctxT"] = _bf(ctx[core // 4].T.copy())
        in_maps.append(im)
    return in_maps


_PROGRAM_CACHE = {}


def run(inputs, debug=False, trace=False):
    key = "dbg" if debug else "plain"
    if key not in _PROGRAM_CACHE:
        _PROGRAM_CACHE[key] = build_program(debug=debug)
    nc = _PROGRAM_CACHE[key]
    in_maps = prepare_inputs(inputs)
    from concourse.bass_utils import run_bass_kernel_spmd
    res = run_bass_kernel_spmd(nc, in_maps, list(range(NCORES)), trace=trace)
    outs = res.results
    full = np.empty((B * H * W, T, C), np.float32)
    for core in range(NCORES):
        full[NR * core:NR * (core + 1)] = outs[core]["out"]
    y = full.reshape(B, H, W, T, C).transpose(0, 4, 3, 1, 2)
    return y, res, outs


def kernel(**inputs):
    y, _, _ = run(inputs)
    return y.astype(np.float32)



# revision 31
# speedup vs baseline: 1.9300x; 1.9300x over previous
"""Trainium2 Bass kernel for BasicTransformerBlockST (spatial/temporal block).

Sharding over 8 NeuronCores (same as baseline):
  Phase A (spatial self-attn): data-parallel over (b,t): core i owns the 4
  groups bt = i + 8g. An 8-way AllToAll (split in two, overlapped with phase
  A compute) reshards to (b,h,w)-parallel: core j owns rows
  (b=j//4, hw in [144*(j%4), 144*(j%4+1))), tokens r-major (token = r*16+t).
  t1 / cross-attn / t2 / FFN run on that shard with the residual stream
  resident in SBUF (no DRAM bounces).

Optimized for the TimelineSim cost model: batched big instructions, S^T
softmax formulation (no attention-matrix transposes or renormalize in phase
A / cross), z via ones-column fused into AV, evictions spread across
DVE/Act/Pool, PSUM tag sharing for double buffering.
"""

import sys

sys.path.insert(0, "/opt/trn_rl_repo")

import numpy as np
import ml_dtypes

import concourse.bass as bass
import concourse.bacc as bacc
import concourse.mybir as mybir
import concourse.tile as tile
from concourse.masks import make_identity

F32 = mybir.dt.float32
BF16 = mybir.dt.bfloat16
AF = mybir.ActivationFunctionType
ALU = mybir.AluOpType
AX = mybir.AxisListType

B, C, T, H, W = 2, 640, 16, 24, 24
HEADS, DH = 8, 80
CTXD = 1024
MAXREL = 16
NREL = 2 * MAXREL + 1          # 33
FFI = 4 * C                    # 2560
INNER = HEADS * DH             # 640
SCALE = DH ** -0.5
EPS = 1e-5

NCORES = 8
NG = 4                         # spatial groups per core
SEQ = H * W                    # 576
NR = (B * H * W) // NCORES     # 144 rows per core
TOK = NR * T                   # 2304 tokens per core
NWIN = TOK // 128              # 18
CHUNKS = C // 128              # 5
CTXCH = CTXD // 128            # 8
HALFW = NWIN // 2              # 9 windows per temporal half
HR = NR // 2                   # 72 rows per half
HTOK = 128 * HALFW             # 1152 tokens per half
NG2 = FFI // 128               # 20 ffn chunks

# token chunks of a 576-token spatial group
QSP = [(0, 128), (128, 128), (256, 128), (384, 128), (512, 64)]


def build_program(debug=False):
    nc = bacc.Bacc(None, target_bir_lowering=False)

    xs_in = nc.dram_tensor("xs_in", [NG, SEQ, C], F32, kind="ExternalInput")
    ctxT_in = nc.dram_tensor("ctxT", [CTXD, 77], BF16, kind="ExternalInput")

    def win(name, shape, dt=BF16):
        return nc.dram_tensor(name, shape, dt, kind="ExternalInput")

    wts = {}
    for p in ("a1", "a2", "t1", "t2"):
        cin = CTXD if p == "a2" else C
        wts[f"{p}_wq"] = win(f"{p}_wq", [C, INNER])
        wts[f"{p}_wk"] = win(f"{p}_wk", [cin, INNER])
        wts[f"{p}_wv"] = win(f"{p}_wv", [cin, INNER])
        wts[f"{p}_wo"] = win(f"{p}_wo", [DH, HEADS, C])
    for p in ("t1", "t2"):
        wts[f"{p}_rkT"] = win(f"{p}_rkT", [DH, NREL])
        wts[f"{p}_rvs"] = win(f"{p}_rvs", [16, T, DH])  # rvs[j,t,d]=rv[j-t+16,d]
    wts["ff_w1"] = win("ff_w1", [C, 2 * FFI])  # host-permuted cols (4a,4g)
    wts["ff_w2"] = win("ff_w2", [FFI, C])
    bd_mask = win("bd_mask", [128, 128], BF16)

    out_final = nc.dram_tensor("out", [NR, T, C], F32, kind="ExternalOutput")
    dbg = {}
    if debug:
        dbg["a"] = nc.dram_tensor("dbg_a", [NG, SEQ, C], F32, kind="ExternalOutput")
        for nm in ("t1", "x2", "t2"):
            dbg[nm] = nc.dram_tensor(f"dbg_{nm}", [NR, T, C], F32,
                                     kind="ExternalOutput")
        dbg["aG"] = nc.dram_tensor("dbg_aG", [128, HEADS, 128], BF16,
                                   kind="ExternalOutput")
        dbg["v0"] = nc.dram_tensor("dbg_v0", [128, C], BF16,
                                   kind="ExternalOutput")
        dbg["q0"] = nc.dram_tensor("dbg_q0", [DH, HEADS, 128], BF16,
                                   kind="ExternalOutput")
        dbg["oT0"] = nc.dram_tensor("dbg_oT0", [DH, HEADS, 128], BF16,
                                    kind="ExternalOutput")

    # slot-major a2a: slot s holds frames t = i + 8*s from src core i
    a2a_in = nc.dram_tensor("a2a_in", [2, NCORES, NR, C], F32)
    a2a_out = nc.dram_tensor("a2a_out", [2, NCORES, NR, C], F32)
    s2_dram = nc.dram_tensor("s2_dram", [TOK, HEADS, 16], BF16)
    groups = [[0, 1, 2, 3, 4, 5, 6, 7]]

    from contextlib import ExitStack

    with tile.TileContext(nc) as tc, ExitStack() as top:
        const = top.enter_context(tc.tile_pool(name="const", bufs=1))
        identb = const.tile([128, 128], BF16)
        make_identity(nc, identb)
        eps_t = const.tile([128, 1], F32)
        nc.vector.memset(eps_t[:], EPS)
        mask = const.tile([128, 128], BF16)
        nc.sync.dma_start(out=mask[:], in_=bd_mask[:, :])
        small = top.enter_context(tc.tile_pool(name="small", bufs=6))
        resp = top.enter_context(tc.tile_pool(name="resp", bufs=1))
        x_sb = resp.tile([128, NWIN, C], F32, tag="x_sb")

        ev_state = [0]

        def evict(out, in_, w=(1, 1, 1)):
            """psum->sbuf copy via rotating engines; w=(dve, act, act2).
            GPSIMD cannot access PSUM, so only DVE/Act here."""
            seq = [nc.vector] * w[0] + [nc.scalar] * (w[1] + w[2])
            eng = seq[ev_state[0] % len(seq)]
            ev_state[0] += 1
            if eng is nc.scalar:
                eng.copy(out=out, in_=in_)
            else:
                eng.tensor_copy(out=out, in_=in_)

        def ln_fm(psp, zp, xfn, zT, nw):
            """LayerNorm (g/b folded into weights) + transpose into
            feature-major zT[:, ci, 128*w : 128*w+128] bf16."""
            zT_a = zT[:, :, :]
            ntok = zT_a.ap[1][0]
            for w in range(nw):
                x = xfn(w)
                st = small.tile([128, 2, 6], F32, tag="bnst")
                nc.vector.bn_stats(out=st[:, 0, :], in_=x[:, 0:512])
                nc.vector.bn_stats(out=st[:, 1, :], in_=x[:, 512:640])
                mv = small.tile([128, 2], F32, tag="bnmv")
                nc.vector.bn_aggr(out=mv[:], in_=st[:])
                rstd = small.tile([128, 1], F32, tag="rstd")
                nc.scalar.activation(out=rstd[:], in_=mv[:, 1:2], func=AF.Sqrt,
                                     bias=eps_t[:], scale=1.0)
                nc.vector.reciprocal(out=rstd[:], in_=rstd[:])
                zs = zp.tile([128, C], BF16, tag="zs")
                nc.vector.tensor_scalar(
                    out=zs[:], in0=x, scalar1=mv[:, 0:1], scalar2=rstd[:],
                    op0=ALU.subtract, op1=ALU.mult)
                pz = psp.tile([128, CHUNKS, 128], BF16, tag="pz")
                for c in range(CHUNKS):
                    nc.tensor.transpose(pz[:, c, :], zs[:, 128 * c:128 * (c + 1)],
                                        identb[:])
                dst = bass.AP(tensor=zT.tensor,
                              offset=zT_a.offset + 128 * w,
                              ap=[list(zT_a.ap[0]), [ntok, CHUNKS], [1, 128]])
                evict(dst, pz[:, :, :], w=(2, 1, 1))

        def load_w_cin(wp, name, cin):
            t = wp.tile([128, cin // 128, wts[name].shape[-1]], BF16,
                        tag="w_" + name)
            nc.sync.dma_start(out=t[:],
                              in_=wts[name][:].rearrange("(a p) n -> p a n", p=128))
            return t

        def load_wo(wp, name):
            t = wp.tile([DH, HEADS, C], BF16, tag="w_" + name)
            nc.sync.dma_start(out=t[:], in_=wts[name][:])
            return t

        def proj_fm(psp, zT, w_sb, qT, ntok):
            """feature-major projection qT[80, h, ntok] (bf16).
            PSUM allocations cap at 4KB, so one 1-bank tile per 512-split."""
            for h in range(HEADS):
                for o in range(0, ntok, 512):
                    n = min(512, ntok - o)
                    pq = psp.tile([128, 512], F32, tag="pA")
                    for ci in range(CHUNKS):
                        nc.tensor.matmul(pq[:DH, 0:n],
                                         w_sb[:, ci, DH * h:DH * (h + 1)],
                                         zT[:, ci, o:o + n],
                                         start=(ci == 0), stop=(ci == CHUNKS - 1))
                    evict(qT[:, h, o:o + n], pq[:DH, 0:n], w=(2, 2, 1))

        def wo_resid(psp, tag, oT, qoff, ntok, wo_sb, resid_ap):
            """WO projection (by-head lhsT oT[:, h, qoff:qoff+ntok]) +
            residual add into resid_ap [ntok, C]."""
            pw = psp.tile([128, 1024], F32, tag=tag)
            for o, n in ((0, 512), (512, 128)):
                for h in range(HEADS):
                    nc.tensor.matmul(pw[:ntok, o:o + n],
                                     oT[:, h, qoff:qoff + ntok],
                                     wo_sb[:, h, o:o + n],
                                     start=(h == 0), stop=(h == HEADS - 1))
            nc.vector.scalar_tensor_tensor(
                out=resid_ap, in0=pw[:ntok, 0:C], scalar=1.0, in1=resid_ap,
                op0=ALU.mult, op1=ALU.add)

        # =====================================================================
        # PHASE A: spatial self-attention per (b,t) group; order 0,2,1,3 so
        # each a2a slot's collective fires after two groups.
        # =====================================================================
        with ExitStack() as ph:
            wp = ph.enter_context(tc.tile_pool(name="wpA", bufs=1))
            zp = ph.enter_context(tc.tile_pool(name="zpA", bufs=2))
            qp = ph.enter_context(tc.tile_pool(name="qpA", bufs=2))
            ap_ = ph.enter_context(tc.tile_pool(name="apA", bufs=2))
            psp = ph.enter_context(tc.tile_pool(name="psA", bufs=2, space="PSUM"))
            pso = ph.enter_context(tc.tile_pool(name="psoA", bufs=1, space="PSUM"))

            wq = load_w_cin(wp, "a1_wq", C)
            wk = load_w_cin(wp, "a1_wk", C)
            wv = load_w_cin(wp, "a1_wv", C)
            wo = load_wo(wp, "a1_wo")

            for g in (0, 2, 1, 3):
                xg = zp.tile([128, CHUNKS, C], F32, tag="xa")
                nc.sync.dma_start(out=xg[:, 0:4, :],
                                  in_=xs_in[g, 0:512, :].rearrange(
                                      "(a p) c -> p a c", p=128))
                nc.sync.dma_start(out=xg[:64, 4, :], in_=xs_in[g, 512:576, :])

                zT = zp.tile([128, CHUNKS, 640], BF16, tag="zTa")
                ln_fm(psp, zp, lambda w: xg[:, w, :], zT, 5)

                qT = qp.tile([DH, HEADS, SEQ], BF16, tag="qa")
                kT = qp.tile([DH, HEADS, SEQ], BF16, tag="ka")
                proj_fm(psp, zT, wq, qT, SEQ)
                proj_fm(psp, zT, wk, kT, SEQ)

                # v token-major with ones column per head (memset 1.0 first;
                # the projection evictions overwrite all but the ones column)
                v1 = qp.tile([128, CHUNKS, HEADS, DH + 1], BF16, tag="va")
                nc.gpsimd.memset(v1[:], 1.0)
                for (w, (o_, np_)) in enumerate(QSP):
                    pv = psp.tile([128, 1024], F32, tag="pA")
                    for o, n in ((0, 512), (512, 128)):
                        for ci in range(CHUNKS):
                            nc.tensor.matmul(pv[:np_, o:o + n],
                                             zT[:, ci, o_:o_ + np_],
                                             wv[:, ci, o:o + n],
                                             start=(ci == 0), stop=(ci == CHUNKS - 1))
                    v1a = v1[:, :, :, :]
                    dst = bass.AP(tensor=v1.tensor,
                                  offset=v1a.offset + w * HEADS * (DH + 1),
                                  ap=[[v1a.ap[0][0], np_], [DH + 1, HEADS],
                                      [1, DH]])
                    evict(dst, pv[:np_, 0:C], w=(2, 1, 1))

                oT = ap_.tile([DH, HEADS, SEQ], BF16, tag="oa")
                for h in range(HEADS):
                    eS = ap_.tile([128, CHUNKS, SEQ], BF16, tag="eS")
                    for (kc, (ko, kp)) in enumerate(QSP):
                        ps = psp.tile([128, 1024], F32, tag="pA")
                        for o, n in ((0, 512), (512, 64)):
                            nc.tensor.matmul(ps[:kp, o:o + n],
                                             kT[:, h, ko:ko + kp],
                                             qT[:, h, o:o + n],
                                             start=True, stop=True)
                        nc.scalar.activation(out=eS[:kp, kc, 0:SEQ],
                                             in_=ps[:kp, 0:SEQ],
                                             func=AF.Exp, scale=SCALE)
                    # AV + z via ones column: oA[q, 80] = z
                    oA = pso.tile([128, CHUNKS, 96], F32, tag="oA")
                    for (qc, (qo, qp_)) in enumerate(QSP):
                        for (kc, (ko, kp)) in enumerate(QSP):
                            nc.tensor.matmul(oA[:qp_, qc, 0:DH + 1],
                                             eS[:kp, kc, qo:qo + qp_],
                                             v1[:kp, kc, h, :],
                                             start=(kc == 0), stop=(kc == 4))
                    rz = small.tile([128, CHUNKS], F32, tag="rz")
                    oAa = oA[:, :, :]
                    zv = bass.AP(tensor=oA.tensor, offset=oAa.offset + DH,
                                 ap=[list(oAa.ap[0]), [96, CHUNKS]])
                    nc.vector.reciprocal(out=rz[:], in_=zv)
                    oN = ap_.tile([128, CHUNKS, DH], BF16, tag="oN")
                    src = bass.AP(tensor=oA.tensor, offset=oAa.offset,
                                  ap=[list(oAa.ap[0]), [96, CHUNKS], [1, DH]])
                    rza = rz[:, :]
                    rzb = bass.AP(tensor=rz.tensor, offset=rza.offset,
                                  ap=[list(rza.ap[0]), [1, CHUNKS], [0, DH]])
                    nc.vector.tensor_tensor(out=oN[:], in0=src, in1=rzb,
                                            op=ALU.mult)
                    pt = pso.tile([DH, CHUNKS, 128], BF16, tag="pt")
                    for (qc, (qo, qp_)) in enumerate(QSP):
                        nc.tensor.transpose(pt[:, qc, 0:qp_], oN[:qp_, qc, :],
                                            identb[:qp_, :qp_])
                    pta = pt[:, :, :]
                    src = bass.AP(tensor=pt.tensor, offset=pta.offset,
                                  ap=[list(pta.ap[0]), [128, 4], [1, 128]])
                    evict(oT[:, h, 0:512], src, w=(2, 1, 1))
                    evict(oT[:, h, 512:576], pt[:, 4, 0:64], w=(2, 1, 1))

                # WO + residual (in place on xg) + scatter to a2a_in
                b_, tslot = g // 2, g % 2
                for (qc, (qo, qp_)) in enumerate(QSP):
                    xq = xg[:qp_, qc, :]
                    wo_resid(psp, "pA", oT, qo, qp_, wo, xq)
                    q0, q1 = qo // NR, (qo + qp_ - 1) // NR
                    for q in range(q0, q1 + 1):
                        lo, hi = max(qo, NR * q), min(qo + qp_, NR * (q + 1))
                        nc.sync.dma_start(
                            out=a2a_in[tslot, 4 * b_ + q, lo - NR * q:hi - NR * q, :],
                            in_=xg[lo - qo:hi - qo, qc, :])
                    if debug:
                        nc.sync.dma_start(out=dbg["a"][g, qo:qo + qp_, :],
                                          in_=xg[:qp_, qc, :])
                if g == 2:
                    nc.gpsimd.collective_compute(
                        "AllToAll", ALU.bypass, replica_groups=groups,
                        ins=[a2a_in[0]], outs=[a2a_out[0]])
            nc.gpsimd.collective_compute(
                "AllToAll", ALU.bypass, replica_groups=groups,
                ins=[a2a_in[1]], outs=[a2a_out[1]])

        # fill x_sb windows from a2a_out: partition p=16r'+t, t=i+8s
        base = a2a_out[:]
        for w in range(NWIN):
            src = bass.AP(tensor=base.tensor,
                          offset=base.offset + 8 * w * C,
                          ap=[[C, 8], [NCORES * NR * C, 2], [NR * C, 8], [1, C]])
            nc.sync.dma_start(out=x_sb[:, w, :], in_=src)

        # =====================================================================
        # Temporal attention (t1 / t2), per half
        # =====================================================================
        def temporal(prefix, dbg_key):
            with ExitStack() as ph:
                wp = ph.enter_context(tc.tile_pool(name="wpT", bufs=1))
                zp = ph.enter_context(tc.tile_pool(name="zpT", bufs=2))
                qp = ph.enter_context(tc.tile_pool(name="qpT", bufs=1))
                sp2 = ph.enter_context(tc.tile_pool(name="sp2T", bufs=2))

                wq = load_w_cin(wp, f"{prefix}_wq", C)
                wk = load_w_cin(wp, f"{prefix}_wk", C)
                wv = load_w_cin(wp, f"{prefix}_wv", C)
                wo = load_wo(wp, f"{prefix}_wo")
                rkT = wp.tile([DH, NREL], BF16, tag="rkT")
                nc.sync.dma_start(out=rkT[:], in_=wts[f"{prefix}_rkT"][:])
                rvs = wp.tile([16, T, DH], BF16, tag="rvs")
                nc.sync.dma_start(out=rvs[:], in_=wts[f"{prefix}_rvs"][:])

                for half in range(2):
                    wlo = half * HALFW
                    zT = zp.tile([128, CHUNKS, HTOK], BF16, tag="zTt")
                    with ExitStack() as hs:
                        psz = hs.enter_context(
                            tc.tile_pool(name="pszT", bufs=2, space="PSUM"))
                        ln_fm(psz, zp, lambda w: x_sb[:, wlo + w, :], zT, HALFW)
                    qT = qp.tile([DH, HEADS, HTOK], BF16, tag="qt")
                    kT = qp.tile([DH, HEADS, HTOK], BF16, tag="kt")
                    with ExitStack() as hs:
                        psq = hs.enter_context(
                            tc.tile_pool(name="psqT", bufs=2, space="PSUM"))
                        proj_fm(psq, zT, wq, qT, HTOK)
                        proj_fm(psq, zT, wk, kT, HTOK)
                    v = qp.tile([128, HALFW, C], BF16, tag="vt")
                    with ExitStack() as hs:
                        psv = hs.enter_context(
                            tc.tile_pool(name="psvT", bufs=2, space="PSUM"))
                        for w in range(HALFW):
                            pv = psv.tile([128, 640], F32, tag="pv")
                            for o, n in ((0, 512), (512, 128)):
                                for ci in range(CHUNKS):
                                    nc.tensor.matmul(
                                        pv[:, o:o + n],
                                        zT[:, ci, 128 * w:128 * (w + 1)],
                                        wv[:, ci, o:o + n],
                                        start=(ci == 0), stop=(ci == CHUNKS - 1))
                            evict(v[:, w, :], pv[:], w=(2, 1, 1))
                    # rel-pos shear: s2byT[r, t, h, j] = q_{r,t}.rk[j-t+16]
                    s2byT = sp2.tile([HR, T, HEADS, 16], BF16, tag="s2byT")
                    with ExitStack() as hs:
                        psh = hs.enter_context(
                            tc.tile_pool(name="pshT", bufs=2, space="PSUM"))
                        for h in range(HEADS):
                            pSB = zp.tile([NREL, HTOK], BF16, tag="pSB")
                            for o in range(0, HTOK, 512):
                                n = min(512, HTOK - o)
                                pp = psh.tile([NREL, 512], F32, tag="pp")
                                nc.tensor.matmul(pp[:, 0:n], rkT[:],
                                                 qT[:, h, o:o + n],
                                                 start=True, stop=True)
                                evict(pSB[:, o:o + n], pp[:, 0:n], w=(1, 1, 1))
                            pSa = pSB[:, :]
                            sh = psh.tile([HR, T, 64], BF16, tag="sh")
                            for t in range(T):
                                src = bass.AP(
                                    tensor=pSB.tensor, offset=pSa.offset + t,
                                    ap=[list(pSa.ap[0]), [16, HR]])
                                nc.tensor.transpose(sh[:, t, 0:NREL], src,
                                                    identb[:NREL, :NREL])
                            # sheared copy: col j of (r,t) = sh[r, t, 16-t+j]
                            sha = sh[:, :, :]
                            s2a = s2byT[:, :, :, :]
                            src = bass.AP(
                                tensor=sh.tensor, offset=sha.offset + 16,
                                ap=[list(sha.ap[0]), [63, 16], [1, 16]])
                            dst = bass.AP(
                                tensor=s2byT.tensor,
                                offset=s2a.offset + 16 * h,
                                ap=[list(s2a.ap[0]), [HEADS * 16, 16], [1, 16]])
                            evict(dst, src, w=(1, 1, 1))
                        # bounce via DRAM: s2_dram[(72*half+r)*16+t, h, j]
                        s2flat = s2_dram[:]
                        d_dst = bass.AP(tensor=s2flat.tensor,
                                        offset=s2flat.offset + half * HR * 2048,
                                        ap=[[2048, HR], [1, 2048]])
                        s2a = s2byT[:, :, :, :]
                        d_src = bass.AP(tensor=s2byT.tensor, offset=s2a.offset,
                                        ap=[list(s2a.ap[0]), [1, 2048]])
                        nc.sync.dma_start(out=d_dst, in_=d_src)

                    # per-window attention, software-pipelined:
                    # fa(w) = scores; back(w-1) = WO+resid; fb(w) = softmax+AV
                    with ExitStack() as hs:
                        psA = hs.enter_context(
                            tc.tile_pool(name="psAT", bufs=2, space="PSUM"))
                        psB = hs.enter_context(
                            tc.tile_pool(name="psBT", bufs=2, space="PSUM"))
                        psC = hs.enter_context(
                            tc.tile_pool(name="psCT", bufs=1, space="PSUM"))
                        s2pitch = T * HEADS * 16

                        def t_fa(w):
                            wg = wlo + w
                            s2w = zp.tile([128, HEADS, 16], BF16, tag="s2w")
                            nc.sync.dma_start(
                                out=s2w[:], in_=s2_dram[128 * wg:128 * (wg + 1)])
                            # emask = mask * exp(scale*s2w), built on Act/Pool
                            # off the critical path
                            eb = zp.tile([128, HEADS, 16], BF16, tag="eb")
                            nc.scalar.activation(out=eb[:], in_=s2w[:],
                                                 func=AF.Exp, scale=SCALE)
                            em = zp.tile([128, HEADS, 128], BF16, tag="em")
                            eba = eb[:, :, :]
                            for h in range(HEADS):
                                ebr = bass.AP(tensor=eb.tensor,
                                              offset=eba.offset + 16 * h,
                                              ap=[list(eba.ap[0]), [0, 8],
                                                  [1, 16]])
                                nc.gpsimd.tensor_tensor(out=em[:, h, :],
                                                        in0=mask[:, :], in1=ebr,
                                                        op=ALU.mult)
                            pS = psA.tile([128, 1024], F32, tag="pS")
                            for h in range(HEADS):
                                nc.tensor.matmul(
                                    pS[:, 128 * h:128 * (h + 1)],
                                    qT[:, h, 128 * w:128 * (w + 1)],
                                    kT[:, h, 128 * w:128 * (w + 1)],
                                    start=True, stop=True)
                            return pS, em

                        def t_fb(w, pS, em):
                            aG = zp.tile([128, HEADS, 128], BF16, tag="aG")
                            nc.scalar.activation(out=aG[:], in_=pS[:],
                                                 func=AF.Exp, scale=SCALE)
                            nc.vector.tensor_tensor(out=aG[:], in0=aG[:],
                                                    in1=em[:], op=ALU.mult)
                            aD = zp.tile([128, HEADS, 16], F32, tag="aD")
                            aGa = aG[:, :, :]
                            agv = bass.AP(tensor=aG.tensor, offset=aGa.offset,
                                          ap=[list(aGa.ap[0]), [128, HEADS],
                                              [1, 16], [16, 8]])
                            nc.vector.tensor_reduce(out=aD[:], in_=agv,
                                                    axis=AX.X, op=ALU.add)
                            zt = small.tile([128, HEADS], F32, tag="zt")
                            nc.vector.tensor_reduce(out=zt[:], in_=aD[:],
                                                    axis=AX.X, op=ALU.add)
                            nc.vector.reciprocal(out=zt[:], in_=zt[:])
                            zta = zt[:, :]
                            rzb = bass.AP(tensor=zt.tensor, offset=zta.offset,
                                          ap=[list(zta.ap[0]), [1, HEADS],
                                              [0, 128]])
                            nc.vector.tensor_tensor(out=aG[:], in0=aG[:],
                                                    in1=rzb, op=ALU.mult)
                            if debug and prefix == "t1" and wlo + w == 0:
                                nc.sync.dma_start(out=dbg["aG"][:], in_=aG[:])
                                nc.sync.dma_start(out=dbg["v0"][:],
                                                  in_=v[:, 0, :])
                                nc.sync.dma_start(out=dbg["q0"][:],
                                                  in_=qT[:, :, 0:128])
                            rzb2 = bass.AP(tensor=zt.tensor, offset=zta.offset,
                                           ap=[list(zta.ap[0]), [1, HEADS],
                                               [0, 16]])
                            aDn = zp.tile([128, HEADS, 16], BF16, tag="aDn")
                            nc.gpsimd.tensor_tensor(out=aDn[:], in0=aD[:],
                                                    in1=rzb2, op=ALU.mult)
                            paT = psB.tile([128, 1024], BF16, tag="ptr")
                            for h in range(HEADS):
                                nc.tensor.transpose(
                                    paT[:, 128 * h:128 * (h + 1)], aG[:, h, :],
                                    identb[:])
                            aTs = zp.tile([128, HEADS, 128], BF16, tag="aTs")
                            evict(aTs[:], paT[:], w=(2, 1, 0))
                            pdT = psB.tile([128, 1024], BF16, tag="ptr")
                            for h in range(HEADS):
                                nc.tensor.transpose(
                                    pdT[:16, 128 * h:128 * (h + 1)],
                                    aDn[:, h, :], identb[:])
                            aDT = zp.tile([16, HEADS, 128], BF16, tag="aDT")
                            evict(aDT[:], pdT[:16, :], w=(1, 1, 0))
                            # o1 = v^T A (plain start/stop groups per slot)
                            pO = psA.tile([128, 1024], F32, tag="pS")
                            for h in range(HEADS):
                                nc.tensor.matmul(pO[:DH, 128 * h:128 * (h + 1)],
                                                 v[:, w, DH * h:DH * (h + 1)],
                                                 aTs[:, h, :],
                                                 start=True, stop=True)
                            # o2: disjoint strided cols, own psum, no accum
                            pR = psC.tile([128, 1024], F32, tag="po2")
                            aDa = aDT[:, :, :]
                            pRa = pR[:, :]
                            for t in range(T):
                                for h in range(HEADS):
                                    rhs = bass.AP(
                                        tensor=aDT.tensor,
                                        offset=aDa.offset + 128 * h + t,
                                        ap=[list(aDa.ap[0]), [16, 8]])
                                    ov = bass.AP(
                                        tensor=pR.tensor,
                                        offset=pRa.offset + 128 * h + t,
                                        ap=[[pRa.ap[0][0], DH], [16, 8]])
                                    nc.tensor.matmul(ov, rvs[:, t, :], rhs,
                                                     start=True, stop=True)
                            oT = zp.tile([DH, HEADS, 128], BF16, tag="oTt")
                            pOa = pO[:, :]
                            src0 = bass.AP(tensor=pO.tensor, offset=pOa.offset,
                                           ap=[[pOa.ap[0][0], DH], [128, HEADS],
                                               [1, 128]])
                            src1 = bass.AP(tensor=pR.tensor, offset=pRa.offset,
                                           ap=[[pRa.ap[0][0], DH], [128, HEADS],
                                               [1, 128]])
                            nc.scalar.copy(out=oT[:, :, :], in_=src0)
                            nc.vector.tensor_tensor(out=oT[:, :, :], in0=src1,
                                                    in1=oT[:, :, :], op=ALU.add)
                            if debug and prefix == "t1" and wlo + w == 0:
                                nc.sync.dma_start(out=dbg["oT0"][:], in_=oT[:])
                            return oT

                        def t_back(w, oT):
                            wg = wlo + w
                            wo_resid(psA, "pS", oT, 0, 128, wo, x_sb[:, wg, :])
                            if debug:
                                nc.sync.dma_start(
                                    out=dbg[dbg_key][:].rearrange(
                                        "r t c -> (r t) c")[128 * wg:128 * (wg + 1), :],
                                    in_=x_sb[:, wg, :])

                        prev = None
                        for w in range(HALFW):
                            pS, em = t_fa(w)
                            if prev is not None:
                                t_back(w - 1, prev)
                            prev = t_fb(w, pS, em)
                        t_back(HALFW - 1, prev)

        temporal("t1", "t1")

        # =====================================================================
        # Cross-attention
        # =====================================================================
        with ExitStack() as ph:
            wp = ph.enter_context(tc.tile_pool(name="wpX", bufs=1))
            zp = ph.enter_context(tc.tile_pool(name="zpX", bufs=2))
            qp = ph.enter_context(tc.tile_pool(name="qpX", bufs=2))

            wqx = load_w_cin(wp, "a2_wq", C)
            wkc = load_w_cin(wp, "a2_wk", CTXD)
            wvc = load_w_cin(wp, "a2_wv", CTXD)
            wox = load_wo(wp, "a2_wo")
            ctx_sb = wp.tile([128, CTXCH, 77], BF16, tag="ctx")
            nc.sync.dma_start(out=ctx_sb[:],
                              in_=ctxT_in[:].rearrange("(a p) m -> p a m", p=128))

            with ExitStack() as hs:
                psk = hs.enter_context(tc.tile_pool(name="pskX", bufs=2,
                                                    space="PSUM"))
                kctxT = wp.tile([DH, HEADS, 77], BF16, tag="kctx")
                pk = psk.tile([DH, HEADS, 128], F32, tag="pk")
                for h in range(HEADS):
                    for ci in range(CTXCH):
                        nc.tensor.matmul(pk[:, h, 0:77],
                                         wkc[:, ci, DH * h:DH * (h + 1)],
                                         ctx_sb[:, ci, :],
                                         start=(ci == 0), stop=(ci == CTXCH - 1))
                pka = pk[:, :, :]
                src = bass.AP(tensor=pk.tensor, offset=pka.offset,
                              ap=[list(pka.ap[0]), [128, HEADS], [1, 77]])
                evict(kctxT[:, :, :], src, w=(1, 1, 1))
                v1x = wp.tile([77, HEADS, DH + 1], BF16, tag="vctx")
                nc.gpsimd.memset(v1x[:], 1.0)
                pv = psk.tile([77, 1024], F32, tag="pvx")
                for o, n in ((0, 512), (512, 128)):
                    for ci in range(CTXCH):
                        nc.tensor.matmul(pv[:, o:o + n], ctx_sb[:, ci, :],
                                         wvc[:, ci, o:o + n],
                                         start=(ci == 0), stop=(ci == CTXCH - 1))
                v1a = v1x[:, :, :]
                dst = bass.AP(tensor=v1x.tensor, offset=v1a.offset,
                              ap=[list(v1a.ap[0]), [DH + 1, HEADS], [1, DH]])
                evict(dst, pv[:, 0:C], w=(1, 1, 1))

            for half in range(2):
                wlo = half * HALFW
                zT = zp.tile([128, CHUNKS, HTOK], BF16, tag="zTx")
                qT = qp.tile([DH, HEADS, HTOK], BF16, tag="qx")
                with ExitStack() as hs:
                    psz = hs.enter_context(tc.tile_pool(name="pszX", bufs=2,
                                                        space="PSUM"))
                    ln_fm(psz, zp, lambda w: x_sb[:, wlo + w, :], zT, HALFW)
                    proj_fm(psz, zT, wqx, qT, HTOK)
                with ExitStack() as hs:
                    pss = hs.enter_context(tc.tile_pool(name="pssX", bufs=2,
                                                        space="PSUM"))
                    psB = hs.enter_context(tc.tile_pool(name="psBX", bufs=2,
                                                        space="PSUM"))
                    eS = qp.tile([77, HEADS, HTOK], BF16, tag="eSx")
                    for h in range(HEADS):
                        for o in range(0, HTOK, 512):
                            n = min(512, HTOK - o)
                            ps = pss.tile([77, 512], F32, tag="psx")
                            nc.tensor.matmul(ps[:, 0:n], kctxT[:, h, :],
                                             qT[:, h, o:o + n],
                                             start=True, stop=True)
                            nc.scalar.activation(out=eS[:, h, o:o + n],
                                                 in_=ps[:, 0:n],
                                                 func=AF.Exp, scale=SCALE)
                    def x_fa(w):
                        oX = psB.tile([128, 1024], F32, tag="oX")
                        for h in range(HEADS):
                            nc.tensor.matmul(oX[:, 128 * h:128 * h + DH + 1],
                                             eS[:, h, 128 * w:128 * (w + 1)],
                                             v1x[:, h, :],
                                             start=True, stop=True)
                        return oX

                    def x_fb(w, oX):
                        rz = small.tile([128, HEADS], F32, tag="rzx")
                        oXa = oX[:, :]
                        zv = bass.AP(tensor=oX.tensor, offset=oXa.offset + DH,
                                     ap=[list(oXa.ap[0]), [128, HEADS]])
                        nc.vector.reciprocal(out=rz[:], in_=zv)
                        oN = zp.tile([128, HEADS, DH], BF16, tag="oNx")
                        src = bass.AP(tensor=oX.tensor, offset=oXa.offset,
                                      ap=[list(oXa.ap[0]), [128, HEADS],
                                          [1, DH]])
                        rza = rz[:, :]
                        rzb = bass.AP(tensor=rz.tensor, offset=rza.offset,
                                      ap=[list(rza.ap[0]), [1, HEADS], [0, DH]])
                        nc.vector.tensor_tensor(out=oN[:], in0=src, in1=rzb,
                                                op=ALU.mult)
                        pt = psB.tile([DH, HEADS, 128], BF16, tag="ptx")
                        for h in range(HEADS):
                            nc.tensor.transpose(pt[:, h, :], oN[:, h, :],
                                                identb[:])
                        oTx = zp.tile([DH, HEADS, 128], BF16, tag="oTx")
                        evict(oTx[:], pt[:], w=(2, 1, 0))
                        return oTx

                    def x_back(w, oTx):
                        wg = wlo + w
                        wo_resid(psB, "oX", oTx, 0, 128, wox, x_sb[:, wg, :])
                        if debug:
                            nc.sync.dma_start(
                                out=dbg["x2"][:].rearrange(
                                    "r t c -> (r t) c")[128 * wg:128 * (wg + 1), :],
                                in_=x_sb[:, wg, :])

                    prev = None
                    for w in range(HALFW):
                        oX = x_fa(w)
                        if prev is not None:
                            x_back(w - 1, prev)
                        prev = x_fb(w, oX)
                    x_back(HALFW - 1, prev)

        temporal("t2", "t2")

        # =====================================================================
        # GEGLU FFN per window. ff_w1 cols host-permuted into rounds of
        # (4 a-chunks, 4 gate-chunks); a-chunk order preserved for ff_w2.
        # =====================================================================
        with ExitStack() as ph:
            wp = ph.enter_context(tc.tile_pool(name="wpF", bufs=1))
            zp = ph.enter_context(tc.tile_pool(name="zpF", bufs=2))
            hp = ph.enter_context(tc.tile_pool(name="hpF", bufs=2))
            psp = ph.enter_context(tc.tile_pool(name="psF", bufs=2, space="PSUM"))
            psx = ph.enter_context(tc.tile_pool(name="psxF", bufs=1, space="PSUM"))
            psh = ph.enter_context(tc.tile_pool(name="pshF", bufs=2, space="PSUM"))

            w1 = wp.tile([128, CHUNKS, 2 * FFI], BF16, tag="w1")
            nc.sync.dma_start(out=w1[:],
                              in_=wts["ff_w1"][:].rearrange("(a p) n -> p a n",
                                                            p=128))
            w2 = wp.tile([128, NG2, C], BF16, tag="w2")
            nc.sync.dma_start(out=w2[:],
                              in_=wts["ff_w2"][:].rearrange("(a p) n -> p a n",
                                                            p=128))

            for w in range(NWIN):
                zT = zp.tile([128, CHUNKS, 128], BF16, tag="zTf")
                ln_fm(psp, zp, lambda _: x_sb[:, w, :], zT, 1)
                uT = hp.tile([128, NG2, 128], BF16, tag="uT")
                for r in range(5):
                    ph_ = psh.tile([128, 8, 128], F32, tag="ph")
                    for co in range(8):
                        gcol = 1024 * r + 128 * co
                        for ci in range(CHUNKS):
                            nc.tensor.matmul(ph_[:, co, :],
                                             w1[:, ci, gcol:gcol + 128],
                                             zT[:, ci, :],
                                             start=(ci == 0), stop=(ci == CHUNKS - 1))
                    gl = hp.tile([128, 4, 128], BF16, tag="gelu")
                    nc.scalar.activation(out=gl[:], in_=ph_[:, 4:8, :],
                                         func=AF.Gelu)
                    nc.vector.tensor_tensor(out=uT[:, 4 * r:4 * r + 4, :],
                                            in0=ph_[:, 0:4, :], in1=gl[:],
                                            op=ALU.mult)
                px = psx.tile([128, 1024], F32, tag="px")
                for o, n in ((0, 512), (512, 128)):
                    for ci in range(NG2):
                        nc.tensor.matmul(px[:, o:o + n], uT[:, ci, :],
                                         w2[:, ci, o:o + n],
                                         start=(ci == 0), stop=(ci == NG2 - 1))
                nc.vector.scalar_tensor_tensor(
                    out=x_sb[:, w, :], in0=px[:, 0:C], scalar=1.0,
                    in1=x_sb[:, w, :], op0=ALU.mult, op1=ALU.add)
                nc.sync.dma_start(
                    out=out_final[:].rearrange(
                        "r t c -> (r t) c")[128 * w:128 * (w + 1), :],
                    in_=x_sb[:, w, :])

    if not nc.is_finalized():
        nc.finalize()
    return nc


# ----------------------------------------------------------------------------
# host side
# ----------------------------------------------------------------------------

def _bf(a):
    return np.asarray(a, dtype=ml_dtypes.bfloat16)


def prepare_inputs(inputs):
    f = {k: np.asarray(v, dtype=np.float32) for k, v in inputs.items()}
    shared = {}

    def fold(g, b, wname):
        wf = f[wname]
        bias = f[b] @ wf
        assert np.abs(bias).max() < 1e-6, f"nonzero folded bias for {wname}"
        return f[g][:, None] * wf

    for k in ("a1_bo", "a2_bo", "t1_bo", "t2_bo", "ff_b1", "ff_b2"):
        assert np.abs(f[k]).max() < 1e-6, f"nonzero bias {k} unsupported"

    for p, gk, bk_ in (("a1", "g1", "b1"), ("t1", "g4", "b4"),
                       ("t2", "g5", "b5")):
        for kind in ("wq", "wk", "wv"):
            shared[f"{p}_{kind}"] = _bf(fold(gk, bk_, f"{p}_{kind}"))
    shared["a2_wq"] = _bf(fold("g2", "b2", "a2_wq"))
    shared["a2_wk"] = _bf(f["a2_wk"])
    shared["a2_wv"] = _bf(f["a2_wv"])
    for p in ("a1", "a2", "t1", "t2"):
        shared[f"{p}_wo"] = _bf(
            f[f"{p}_wo"].reshape(HEADS, DH, C).transpose(1, 0, 2))
    for p in ("t1", "t2"):
        shared[f"{p}_rkT"] = _bf(f[f"{p}_rk"].T)
        rv = f[f"{p}_rv"]
        rvs = np.zeros((16, T, DH), np.float32)
        for t in range(T):
            for j in range(16):
                rvs[j, t] = rv[j - t + MAXREL]
        shared[f"{p}_rvs"] = _bf(rvs)
    w1f = fold("g3", "b3", "ff_w1")
    a_, g_ = w1f[:, :FFI], w1f[:, FFI:]
    cols = []
    for r in range(5):
        cols.append(a_[:, 512 * r:512 * (r + 1)])
        cols.append(g_[:, 512 * r:512 * (r + 1)])
    shared["ff_w1"] = _bf(np.concatenate(cols, axis=1))
    shared["ff_w2"] = _bf(f["ff_w2"])
    m = np.zeros((128, 128), np.float32)
    for g in range(8):
        m[16 * g:16 * (g + 1), 16 * g:16 * (g + 1)] = 1.0
    shared["bd_mask"] = _bf(m)

    x = f["x"]
    ctx = f["context"]
    in_maps = []
    for core in range(NCORES):
        im = dict(shared)
        xs = np.empty((NG, SEQ, C), np.float32)
        for g in range(NG):
            bt = core + 8 * g
            b, t = bt // T, bt % T
            xs[g] = x[b, :, t].reshape(C, SEQ).T
        im["xs_in"] = xs
        im["ctxT"] = _bf(ctx[core // 4].T.copy())
        in_maps.append(im)
    return in_maps


_PROGRAM_CACHE = {}


def run(inputs, debug=False, trace=False):
    key = "dbg" if debug else "plain"
    if key not in _PROGRAM_CACHE:
        _PROGRAM_CACHE[key] = build_program(debug=debug)
    nc = _PROGRAM_CACHE[key]
    in_maps = prepare_inputs(inputs)
    from concourse.bass_utils import run_bass_kernel_spmd
    res = run_bass_kernel_spmd(nc, in_maps, list(range(NCORES)), trace=trace)
    outs = res.results
    full = np.empty((B * H * W, T, C), np.float32)
    for core in range(NCORES):
        full[NR * core:NR * (core + 1)] = outs[core]["out"]
    y = full.reshape(B, H, W, T, C).transpose(0, 4, 3, 1, 2)
    return y, res, outs


def kernel(**inputs):
    y, _, _ = run(inputs)
    return y.astype(np.float32)


# revision 45
# speedup vs baseline: 1.9525x; 1.0117x over previous
"""Trainium2 Bass kernel for BasicTransformerBlockST (spatial/temporal block).

Sharding over 8 NeuronCores (same as baseline):
  Phase A (spatial self-attn): data-parallel over (b,t): core i owns the 4
  groups bt = i + 8g. An 8-way AllToAll (split in two, overlapped with phase
  A compute) reshards to (b,h,w)-parallel: core j owns rows
  (b=j//4, hw in [144*(j%4), 144*(j%4+1))), tokens r-major (token = r*16+t).
  t1 / cross-attn / t2 / FFN run on that shard with the residual stream
  resident in SBUF (no DRAM bounces).

Optimized for the TimelineSim cost model: batched big instructions, S^T
softmax formulation (no attention-matrix transposes or renormalize in phase
A / cross), z via ones-column fused into AV, evictions spread across
DVE/Act/Pool, PSUM tag sharing for double buffering.
"""

import sys

sys.path.insert(0, "/opt/trn_rl_repo")

import numpy as np
import ml_dtypes

import concourse.bass as bass
import concourse.bacc as bacc
import concourse.mybir as mybir
import concourse.tile as tile
from concourse.masks import make_identity

F32 = mybir.dt.float32
BF16 = mybir.dt.bfloat16
AF = mybir.ActivationFunctionType
ALU = mybir.AluOpType
AX = mybir.AxisListType

B, C, T, H, W = 2, 640, 16, 24, 24
HEADS, DH = 8, 80
CTXD = 1024
MAXREL = 16
NREL = 2 * MAXREL + 1          # 33
FFI = 4 * C                    # 2560
INNER = HEADS * DH             # 640
SCALE = DH ** -0.5
EPS = 1e-5

NCORES = 8
NG = 4                         # spatial groups per core
SEQ = H * W                    # 576
NR = (B * H * W) // NCORES     # 144 rows per core
TOK = NR * T                   # 2304 tokens per core
NWIN = TOK // 128              # 18
CHUNKS = C // 128              # 5
CTXCH = CTXD // 128            # 8
HALFW = NWIN // 2              # 9 windows per temporal half
HR = NR // 2                   # 72 rows per half
HTOK = 128 * HALFW             # 1152 tokens per half
NG2 = FFI // 128               # 20 ffn chunks

# token chunks of a 576-token spatial group
QSP = [(0, 128), (128, 128), (256, 128), (384, 128), (512, 64)]


def build_program(debug=False):
    nc = bacc.Bacc(None, target_bir_lowering=False)

    xs_in = nc.dram_tensor("xs_in", [NG, SEQ, C], F32, kind="ExternalInput")
    ctxT_in = nc.dram_tensor("ctxT", [CTXD, 77], BF16, kind="ExternalInput")

    def win(name, shape, dt=BF16):
        return nc.dram_tensor(name, shape, dt, kind="ExternalInput")

    wts = {}
    for p in ("a1", "a2", "t1", "t2"):
        cin = CTXD if p == "a2" else C
        wts[f"{p}_wq"] = win(f"{p}_wq", [C, INNER])
        wts[f"{p}_wk"] = win(f"{p}_wk", [cin, INNER])
        wts[f"{p}_wv"] = win(f"{p}_wv", [cin, INNER])
        wts[f"{p}_wo"] = win(f"{p}_wo", [DH, HEADS, C])
    for p in ("t1", "t2"):
        wts[f"{p}_rkT"] = win(f"{p}_rkT", [DH, NREL])
        wts[f"{p}_rvs"] = win(f"{p}_rvs", [16, T, DH])  # rvs[j,t,d]=rv[j-t+16,d]
    wts["ff_w1"] = win("ff_w1", [C, 2 * FFI])  # host-permuted cols (4a,4g)
    wts["ff_w2"] = win("ff_w2", [FFI, C])
    bd_mask = win("bd_mask", [128, 128], BF16)

    out_final = nc.dram_tensor("out", [NR, T, C], F32, kind="ExternalOutput")
    dbg = {}
    if debug:
        dbg["a"] = nc.dram_tensor("dbg_a", [NG, SEQ, C], F32, kind="ExternalOutput")
        for nm in ("t1", "x2", "t2"):
            dbg[nm] = nc.dram_tensor(f"dbg_{nm}", [NR, T, C], F32,
                                     kind="ExternalOutput")
        dbg["aG"] = nc.dram_tensor("dbg_aG", [128, HEADS, 128], BF16,
                                   kind="ExternalOutput")
        dbg["v0"] = nc.dram_tensor("dbg_v0", [128, C], BF16,
                                   kind="ExternalOutput")
        dbg["q0"] = nc.dram_tensor("dbg_q0", [DH, HEADS, 128], BF16,
                                   kind="ExternalOutput")
        dbg["oT0"] = nc.dram_tensor("dbg_oT0", [DH, HEADS, 128], BF16,
                                    kind="ExternalOutput")

    # slot-major a2a: slot s holds frames t = i + 8*s from src core i
    a2a_in = nc.dram_tensor("a2a_in", [2, NCORES, NR, C], F32)
    a2a_out = nc.dram_tensor("a2a_out", [2, NCORES, NR, C], F32)
    s2_dram = nc.dram_tensor("s2_dram", [TOK, HEADS, 16], BF16)
    groups = [[0, 1, 2, 3, 4, 5, 6, 7]]

    from contextlib import ExitStack

    with tile.TileContext(nc) as tc, ExitStack() as top:
        const = top.enter_context(tc.tile_pool(name="const", bufs=1))
        identb = const.tile([128, 128], BF16)
        make_identity(nc, identb)
        eps_t = const.tile([128, 1], F32)
        nc.vector.memset(eps_t[:], EPS)
        mask = const.tile([128, 128], BF16)
        nc.sync.dma_start(out=mask[:], in_=bd_mask[:, :])
        small = top.enter_context(tc.tile_pool(name="small", bufs=6))
        resp = top.enter_context(tc.tile_pool(name="resp", bufs=1))
        x_sb = resp.tile([128, NWIN, C], F32, tag="x_sb")

        ev_state = [0]

        def evict(out, in_, w=(1, 1, 1)):
            """psum->sbuf copy via rotating engines; w=(dve, act, act2).
            GPSIMD cannot access PSUM, so only DVE/Act here."""
            seq = [nc.vector] * w[0] + [nc.scalar] * (w[1] + w[2])
            eng = seq[ev_state[0] % len(seq)]
            ev_state[0] += 1
            if eng is nc.scalar:
                eng.copy(out=out, in_=in_)
            else:
                eng.tensor_copy(out=out, in_=in_)

        def ln_fm(psp, zp, xfn, zT, nw):
            """LayerNorm (g/b folded into weights) + transpose into
            feature-major zT[:, ci, 128*w : 128*w+128] bf16."""
            zT_a = zT[:, :, :]
            ntok = zT_a.ap[1][0]
            for w in range(nw):
                x = xfn(w)
                st = small.tile([128, 2, 6], F32, tag="bnst")
                nc.vector.bn_stats(out=st[:, 0, :], in_=x[:, 0:512])
                nc.vector.bn_stats(out=st[:, 1, :], in_=x[:, 512:640])
                mv = small.tile([128, 2], F32, tag="bnmv")
                nc.vector.bn_aggr(out=mv[:], in_=st[:])
                rstd = small.tile([128, 1], F32, tag="rstd")
                nc.scalar.activation(out=rstd[:], in_=mv[:, 1:2], func=AF.Sqrt,
                                     bias=eps_t[:], scale=1.0)
                nc.vector.reciprocal(out=rstd[:], in_=rstd[:])
                zs = zp.tile([128, C], BF16, tag="zs")
                nc.vector.tensor_scalar(
                    out=zs[:], in0=x, scalar1=mv[:, 0:1], scalar2=rstd[:],
                    op0=ALU.subtract, op1=ALU.mult)
                pz = psp.tile([128, CHUNKS, 128], BF16, tag="pz")
                for c in range(CHUNKS):
                    nc.tensor.transpose(pz[:, c, :], zs[:, 128 * c:128 * (c + 1)],
                                        identb[:])
                dst = bass.AP(tensor=zT.tensor,
                              offset=zT_a.offset + 128 * w,
                              ap=[list(zT_a.ap[0]), [ntok, CHUNKS], [1, 128]])
                evict(dst, pz[:, :, :], w=(2, 1, 1))

        def load_w_cin(wp, name, cin):
            t = wp.tile([128, cin // 128, wts[name].shape[-1]], BF16,
                        tag="w_" + name)
            nc.sync.dma_start(out=t[:],
                              in_=wts[name][:].rearrange("(a p) n -> p a n", p=128))
            return t

        def load_wo(wp, name):
            t = wp.tile([DH, HEADS, C], BF16, tag="w_" + name)
            nc.sync.dma_start(out=t[:], in_=wts[name][:])
            return t

        def proj_fm(psp, zT, w_sb, qT, ntok):
            """feature-major projection qT[80, h, ntok] (bf16).
            PSUM allocations cap at 4KB, so one 1-bank tile per 512-split."""
            for h in range(HEADS):
                for o in range(0, ntok, 512):
                    n = min(512, ntok - o)
                    pq = psp.tile([128, 512], F32, tag="pA")
                    for ci in range(CHUNKS):
                        nc.tensor.matmul(pq[:DH, 0:n],
                                         w_sb[:, ci, DH * h:DH * (h + 1)],
                                         zT[:, ci, o:o + n],
                                         start=(ci == 0), stop=(ci == CHUNKS - 1))
                    evict(qT[:, h, o:o + n], pq[:DH, 0:n], w=(2, 2, 1))

        def wo_resid(psp, tag, oT, qoff, ntok, wo_sb, resid_ap):
            """WO projection (by-head lhsT oT[:, h, qoff:qoff+ntok]) +
            residual add into resid_ap [ntok, C]."""
            pw = psp.tile([128, 1024], F32, tag=tag)
            for o, n in ((0, 512), (512, 128)):
                for h in range(HEADS):
                    nc.tensor.matmul(pw[:ntok, o:o + n],
                                     oT[:, h, qoff:qoff + ntok],
                                     wo_sb[:, h, o:o + n],
                                     start=(h == 0), stop=(h == HEADS - 1))
            nc.vector.scalar_tensor_tensor(
                out=resid_ap, in0=pw[:ntok, 0:C], scalar=1.0, in1=resid_ap,
                op0=ALU.mult, op1=ALU.add)

        # =====================================================================
        # PHASE A: spatial self-attention per (b,t) group; order 0,2,1,3 so
        # each a2a slot's collective fires after two groups.
        # =====================================================================
        with ExitStack() as ph:
            wp = ph.enter_context(tc.tile_pool(name="wpA", bufs=1))
            zp = ph.enter_context(tc.tile_pool(name="zpA", bufs=2))
            qp = ph.enter_context(tc.tile_pool(name="qpA", bufs=2))
            ap_ = ph.enter_context(tc.tile_pool(name="apA", bufs=2))
            psp = ph.enter_context(tc.tile_pool(name="psA", bufs=2, space="PSUM"))
            pso = ph.enter_context(tc.tile_pool(name="psoA", bufs=1, space="PSUM"))

            wq = load_w_cin(wp, "a1_wq", C)
            wk = load_w_cin(wp, "a1_wk", C)
            wv = load_w_cin(wp, "a1_wv", C)
            wo = load_wo(wp, "a1_wo")

            for g in (0, 2, 1, 3):
                xg = zp.tile([128, CHUNKS, C], F32, tag="xa")
                nc.sync.dma_start(out=xg[:, 0:4, :],
                                  in_=xs_in[g, 0:512, :].rearrange(
                                      "(a p) c -> p a c", p=128))
                nc.sync.dma_start(out=xg[:64, 4, :], in_=xs_in[g, 512:576, :])

                zT = zp.tile([128, CHUNKS, 640], BF16, tag="zTa")
                ln_fm(psp, zp, lambda w: xg[:, w, :], zT, 5)

                qT = qp.tile([DH, HEADS, SEQ], BF16, tag="qa")
                kT = qp.tile([DH, HEADS, SEQ], BF16, tag="ka")
                proj_fm(psp, zT, wq, qT, SEQ)
                proj_fm(psp, zT, wk, kT, SEQ)

                # v token-major with ones column per head (memset 1.0 first;
                # the projection evictions overwrite all but the ones column)
                v1 = qp.tile([128, CHUNKS, HEADS, DH + 1], BF16, tag="va")
                nc.gpsimd.memset(v1[:], 1.0)
                for (w, (o_, np_)) in enumerate(QSP):
                    pv = psp.tile([128, 1024], F32, tag="pA")
                    for o, n in ((0, 512), (512, 128)):
                        for ci in range(CHUNKS):
                            nc.tensor.matmul(pv[:np_, o:o + n],
                                             zT[:, ci, o_:o_ + np_],
                                             wv[:, ci, o:o + n],
                                             start=(ci == 0), stop=(ci == CHUNKS - 1))
                    v1a = v1[:, :, :, :]
                    dst = bass.AP(tensor=v1.tensor,
                                  offset=v1a.offset + w * HEADS * (DH + 1),
                                  ap=[[v1a.ap[0][0], np_], [DH + 1, HEADS],
                                      [1, DH]])
                    evict(dst, pv[:np_, 0:C], w=(2, 1, 1))

                oT = ap_.tile([DH, HEADS, SEQ], BF16, tag="oa")

                def a_front(h):
                    """scores exp(S^T) for head h"""
                    eS = ap_.tile([128, CHUNKS, SEQ], BF16, tag="eS")
                    for (kc, (ko, kp)) in enumerate(QSP):
                        ps = psp.tile([128, 1024], F32, tag="pA")
                        for o, n in ((0, 512), (512, 64)):
                            nc.tensor.matmul(ps[:kp, o:o + n],
                                             kT[:, h, ko:ko + kp],
                                             qT[:, h, o:o + n],
                                             start=True, stop=True)
                        nc.scalar.activation(out=eS[:kp, kc, 0:SEQ],
                                             in_=ps[:kp, 0:SEQ],
                                             func=AF.Exp, scale=SCALE)
                    return eS

                def a_back(h, eS):
                    # AV + z via ones column: oA[q, 80] = z
                    oA = pso.tile([128, CHUNKS, 96], F32, tag="oA")
                    for (qc, (qo, qp_)) in enumerate(QSP):
                        for (kc, (ko, kp)) in enumerate(QSP):
                            nc.tensor.matmul(oA[:qp_, qc, 0:DH + 1],
                                             eS[:kp, kc, qo:qo + qp_],
                                             v1[:kp, kc, h, :],
                                             start=(kc == 0), stop=(kc == 4))
                    rz = small.tile([128, CHUNKS], F32, tag="rz")
                    oAa = oA[:, :, :]
                    zv = bass.AP(tensor=oA.tensor, offset=oAa.offset + DH,
                                 ap=[list(oAa.ap[0]), [96, CHUNKS]])
                    nc.vector.reciprocal(out=rz[:], in_=zv)
                    oN = ap_.tile([128, CHUNKS, DH], BF16, tag="oN")
                    src = bass.AP(tensor=oA.tensor, offset=oAa.offset,
                                  ap=[list(oAa.ap[0]), [96, CHUNKS], [1, DH]])
                    rza = rz[:, :]
                    rzb = bass.AP(tensor=rz.tensor, offset=rza.offset,
                                  ap=[list(rza.ap[0]), [1, CHUNKS], [0, DH]])
                    nc.vector.tensor_tensor(out=oN[:], in0=src, in1=rzb,
                                            op=ALU.mult)
                    pt = pso.tile([DH, CHUNKS, 128], BF16, tag="pt")
                    for (qc, (qo, qp_)) in enumerate(QSP):
                        nc.tensor.transpose(pt[:, qc, 0:qp_], oN[:qp_, qc, :],
                                            identb[:qp_, :qp_])
                    pta = pt[:, :, :]
                    src = bass.AP(tensor=pt.tensor, offset=pta.offset,
                                  ap=[list(pta.ap[0]), [128, 4], [1, 128]])
                    evict(oT[:, h, 0:512], src, w=(2, 1, 1))
                    evict(oT[:, h, 512:576], pt[:, 4, 0:64], w=(2, 1, 1))

                # software-pipeline heads: scores(h+1) before AV/norm(h) so
                # the PE never waits on head h's exp chain
                prev_eS = None
                for h in range(HEADS):
                    eS = a_front(h)
                    if prev_eS is not None:
                        a_back(h - 1, prev_eS)
                    prev_eS = eS
                a_back(HEADS - 1, prev_eS)

                # WO + residual (in place on xg) + scatter to a2a_in
                b_, tslot = g // 2, g % 2
                for (qc, (qo, qp_)) in enumerate(QSP):
                    xq = xg[:qp_, qc, :]
                    wo_resid(psp, "pA", oT, qo, qp_, wo, xq)
                    q0, q1 = qo // NR, (qo + qp_ - 1) // NR
                    for q in range(q0, q1 + 1):
                        lo, hi = max(qo, NR * q), min(qo + qp_, NR * (q + 1))
                        nc.sync.dma_start(
                            out=a2a_in[tslot, 4 * b_ + q, lo - NR * q:hi - NR * q, :],
                            in_=xg[lo - qo:hi - qo, qc, :])
                    if debug:
                        nc.sync.dma_start(out=dbg["a"][g, qo:qo + qp_, :],
                                          in_=xg[:qp_, qc, :])
                if g == 2:
                    nc.gpsimd.collective_compute(
                        "AllToAll", ALU.bypass, replica_groups=groups,
                        ins=[a2a_in[0]], outs=[a2a_out[0]])
            nc.gpsimd.collective_compute(
                "AllToAll", ALU.bypass, replica_groups=groups,
                ins=[a2a_in[1]], outs=[a2a_out[1]])

        # fill x_sb windows from a2a_out: partition p=16r'+t, t=i+8s
        base = a2a_out[:]
        for w in range(NWIN):
            src = bass.AP(tensor=base.tensor,
                          offset=base.offset + 8 * w * C,
                          ap=[[C, 8], [NCORES * NR * C, 2], [NR * C, 8], [1, C]])
            nc.sync.dma_start(out=x_sb[:, w, :], in_=src)

        # =====================================================================
        # Temporal attention (t1 / t2), per half
        # =====================================================================
        def temporal(prefix, dbg_key):
            with ExitStack() as ph:
                wp = ph.enter_context(tc.tile_pool(name="wpT", bufs=1))
                zp = ph.enter_context(tc.tile_pool(name="zpT", bufs=2))
                qp = ph.enter_context(tc.tile_pool(name="qpT", bufs=1))
                sp2 = ph.enter_context(tc.tile_pool(name="sp2T", bufs=2))

                wq = load_w_cin(wp, f"{prefix}_wq", C)
                wk = load_w_cin(wp, f"{prefix}_wk", C)
                wv = load_w_cin(wp, f"{prefix}_wv", C)
                wo = load_wo(wp, f"{prefix}_wo")
                rkT = wp.tile([DH, NREL], BF16, tag="rkT")
                nc.sync.dma_start(out=rkT[:], in_=wts[f"{prefix}_rkT"][:])
                rvs = wp.tile([16, T, DH], BF16, tag="rvs")
                nc.sync.dma_start(out=rvs[:], in_=wts[f"{prefix}_rvs"][:])

                for half in range(2):
                    wlo = half * HALFW
                    zT = zp.tile([128, CHUNKS, HTOK], BF16, tag="zTt")
                    with ExitStack() as hs:
                        psz = hs.enter_context(
                            tc.tile_pool(name="pszT", bufs=2, space="PSUM"))
                        ln_fm(psz, zp, lambda w: x_sb[:, wlo + w, :], zT, HALFW)
                    qT = qp.tile([DH, HEADS, HTOK], BF16, tag="qt")
                    kT = qp.tile([DH, HEADS, HTOK], BF16, tag="kt")
                    with ExitStack() as hs:
                        psq = hs.enter_context(
                            tc.tile_pool(name="psqT", bufs=2, space="PSUM"))
                        proj_fm(psq, zT, wq, qT, HTOK)
                        proj_fm(psq, zT, wk, kT, HTOK)
                    v = qp.tile([128, HALFW, C], BF16, tag="vt")
                    with ExitStack() as hs:
                        psv = hs.enter_context(
                            tc.tile_pool(name="psvT", bufs=2, space="PSUM"))
                        for w in range(HALFW):
                            pv = psv.tile([128, 640], F32, tag="pv")
                            for o, n in ((0, 512), (512, 128)):
                                for ci in range(CHUNKS):
                                    nc.tensor.matmul(
                                        pv[:, o:o + n],
                                        zT[:, ci, 128 * w:128 * (w + 1)],
                                        wv[:, ci, o:o + n],
                                        start=(ci == 0), stop=(ci == CHUNKS - 1))
                            evict(v[:, w, :], pv[:], w=(2, 1, 1))
                    # rel-pos shear: s2byT[r, t, h, j] = q_{r,t}.rk[j-t+16]
                    s2byT = sp2.tile([HR, T, HEADS, 16], BF16, tag="s2byT")
                    with ExitStack() as hs:
                        psh = hs.enter_context(
                            tc.tile_pool(name="pshT", bufs=2, space="PSUM"))
                        for h in range(HEADS):
                            pSB = zp.tile([NREL, HTOK], BF16, tag="pSB")
                            for o in range(0, HTOK, 512):
                                n = min(512, HTOK - o)
                                pp = psh.tile([NREL, 512], F32, tag="pp")
                                nc.tensor.matmul(pp[:, 0:n], rkT[:],
                                                 qT[:, h, o:o + n],
                                                 start=True, stop=True)
                                evict(pSB[:, o:o + n], pp[:, 0:n], w=(1, 1, 1))
                            pSa = pSB[:, :]
                            sh = psh.tile([HR, T, 64], BF16, tag="sh")
                            for t in range(T):
                                src = bass.AP(
                                    tensor=pSB.tensor, offset=pSa.offset + t,
                                    ap=[list(pSa.ap[0]), [16, HR]])
                                nc.tensor.transpose(sh[:, t, 0:NREL], src,
                                                    identb[:NREL, :NREL])
                            # sheared copy: col j of (r,t) = sh[r, t, 16-t+j]
                            sha = sh[:, :, :]
                            s2a = s2byT[:, :, :, :]
                            src = bass.AP(
                                tensor=sh.tensor, offset=sha.offset + 16,
                                ap=[list(sha.ap[0]), [63, 16], [1, 16]])
                            dst = bass.AP(
                                tensor=s2byT.tensor,
                                offset=s2a.offset + 16 * h,
                                ap=[list(s2a.ap[0]), [HEADS * 16, 16], [1, 16]])
                            evict(dst, src, w=(1, 1, 1))
                        # bounce via DRAM: s2_dram[(72*half+r)*16+t, h, j]
                        s2flat = s2_dram[:]
                        d_dst = bass.AP(tensor=s2flat.tensor,
                                        offset=s2flat.offset + half * HR * 2048,
                                        ap=[[2048, HR], [1, 2048]])
                        s2a = s2byT[:, :, :, :]
                        d_src = bass.AP(tensor=s2byT.tensor, offset=s2a.offset,
                                        ap=[list(s2a.ap[0]), [1, 2048]])
                        nc.sync.dma_start(out=d_dst, in_=d_src)

                    # per-window attention, software-pipelined:
                    # fa(w) = scores; back(w-1) = WO+resid; fb(w) = softmax+AV
                    with ExitStack() as hs:
                        psA = hs.enter_context(
                            tc.tile_pool(name="psAT", bufs=2, space="PSUM"))
                        psB = hs.enter_context(
                            tc.tile_pool(name="psBT", bufs=2, space="PSUM"))
                        psC = hs.enter_context(
                            tc.tile_pool(name="psCT", bufs=1, space="PSUM"))
                        s2pitch = T * HEADS * 16

                        def t_fa(w):
                            wg = wlo + w
                            s2w = zp.tile([128, HEADS, 16], BF16, tag="s2w")
                            nc.sync.dma_start(
                                out=s2w[:], in_=s2_dram[128 * wg:128 * (wg + 1)])
                            # emask = mask * exp(scale*s2w), built on Act/Pool
                            # off the critical path
                            eb = zp.tile([128, HEADS, 16], BF16, tag="eb")
                            nc.scalar.activation(out=eb[:], in_=s2w[:],
                                                 func=AF.Exp, scale=SCALE)
                            em = zp.tile([128, HEADS, 128], BF16, tag="em")
                            eba = eb[:, :, :]
                            for h in range(HEADS):
                                ebr = bass.AP(tensor=eb.tensor,
                                              offset=eba.offset + 16 * h,
                                              ap=[list(eba.ap[0]), [0, 8],
                                                  [1, 16]])
                                nc.gpsimd.tensor_tensor(out=em[:, h, :],
                                                        in0=mask[:, :], in1=ebr,
                                                        op=ALU.mult)
                            pS = psA.tile([128, 1024], F32, tag="pS")
                            for h in range(HEADS):
                                nc.tensor.matmul(
                                    pS[:, 128 * h:128 * (h + 1)],
                                    qT[:, h, 128 * w:128 * (w + 1)],
                                    kT[:, h, 128 * w:128 * (w + 1)],
                                    start=True, stop=True)
                            return pS, em

                        def t_fb(w, pS, em):
                            aG = zp.tile([128, HEADS, 128], BF16, tag="aG")
                            nc.scalar.activation(out=aG[:], in_=pS[:],
                                                 func=AF.Exp, scale=SCALE)
                            nc.vector.tensor_tensor(out=aG[:], in0=aG[:],
                                                    in1=em[:], op=ALU.mult)
                            aD = zp.tile([128, HEADS, 16], F32, tag="aD")
                            aGa = aG[:, :, :]
                            agv = bass.AP(tensor=aG.tensor, offset=aGa.offset,
                                          ap=[list(aGa.ap[0]), [128, HEADS],
                                              [1, 16], [16, 8]])
                            nc.vector.tensor_reduce(out=aD[:], in_=agv,
                                                    axis=AX.X, op=ALU.add)
                            zt = small.tile([128, HEADS], F32, tag="zt")
                            nc.vector.tensor_reduce(out=zt[:], in_=aD[:],
                                                    axis=AX.X, op=ALU.add)
                            nc.vector.reciprocal(out=zt[:], in_=zt[:])
                            zta = zt[:, :]
                            rzb = bass.AP(tensor=zt.tensor, offset=zta.offset,
                                          ap=[list(zta.ap[0]), [1, HEADS],
                                              [0, 128]])
                            nc.vector.tensor_tensor(out=aG[:], in0=aG[:],
                                                    in1=rzb, op=ALU.mult)
                            if debug and prefix == "t1" and wlo + w == 0:
                                nc.sync.dma_start(out=dbg["aG"][:], in_=aG[:])
                                nc.sync.dma_start(out=dbg["v0"][:],
                                                  in_=v[:, 0, :])
                                nc.sync.dma_start(out=dbg["q0"][:],
                                                  in_=qT[:, :, 0:128])
                            rzb2 = bass.AP(tensor=zt.tensor, offset=zta.offset,
                                           ap=[list(zta.ap[0]), [1, HEADS],
                                               [0, 16]])
                            aDn = zp.tile([128, HEADS, 16], BF16, tag="aDn")
                            nc.gpsimd.tensor_tensor(out=aDn[:], in0=aD[:],
                                                    in1=rzb2, op=ALU.mult)
                            paT = psB.tile([128, 1024], BF16, tag="ptr")
                            for h in range(HEADS):
                                nc.tensor.transpose(
                                    paT[:, 128 * h:128 * (h + 1)], aG[:, h, :],
                                    identb[:])
                            aTs = zp.tile([128, HEADS, 128], BF16, tag="aTs")
                            evict(aTs[:], paT[:], w=(2, 1, 0))
                            pdT = psB.tile([128, 1024], BF16, tag="ptr")
                            for h in range(HEADS):
                                nc.tensor.transpose(
                                    pdT[:16, 128 * h:128 * (h + 1)],
                                    aDn[:, h, :], identb[:])
                            aDT = zp.tile([16, HEADS, 128], BF16, tag="aDT")
                            evict(aDT[:], pdT[:16, :], w=(1, 1, 0))
                            # o1 = v^T A (plain start/stop groups per slot)
                            pO = psA.tile([128, 1024], F32, tag="pS")
                            for h in range(HEADS):
                                nc.tensor.matmul(pO[:DH, 128 * h:128 * (h + 1)],
                                                 v[:, w, DH * h:DH * (h + 1)],
                                                 aTs[:, h, :],
                                                 start=True, stop=True)
                            # o2: disjoint strided cols, own psum, no accum
                            pR = psC.tile([128, 1024], F32, tag="po2")
                            aDa = aDT[:, :, :]
                            pRa = pR[:, :]
                            for t in range(T):
                                for h in range(HEADS):
                                    rhs = bass.AP(
                                        tensor=aDT.tensor,
                                        offset=aDa.offset + 128 * h + t,
                                        ap=[list(aDa.ap[0]), [16, 8]])
                                    ov = bass.AP(
                                        tensor=pR.tensor,
                                        offset=pRa.offset + 128 * h + t,
                                        ap=[[pRa.ap[0][0], DH], [16, 8]])
                                    nc.tensor.matmul(ov, rvs[:, t, :], rhs,
                                                     start=True, stop=True)
                            oT = zp.tile([DH, HEADS, 128], BF16, tag="oTt")
                            pOa = pO[:, :]
                            src0 = bass.AP(tensor=pO.tensor, offset=pOa.offset,
                                           ap=[[pOa.ap[0][0], DH], [128, HEADS],
                                               [1, 128]])
                            src1 = bass.AP(tensor=pR.tensor, offset=pRa.offset,
                                           ap=[[pRa.ap[0][0], DH], [128, HEADS],
                                               [1, 128]])
                            nc.scalar.copy(out=oT[:, :, :], in_=src0)
                            nc.vector.tensor_tensor(out=oT[:, :, :], in0=src1,
                                                    in1=oT[:, :, :], op=ALU.add)
                            if debug and prefix == "t1" and wlo + w == 0:
                                nc.sync.dma_start(out=dbg["oT0"][:], in_=oT[:])
                            return oT

                        def t_back(w, oT):
                            wg = wlo + w
                            wo_resid(psA, "pS", oT, 0, 128, wo, x_sb[:, wg, :])
                            if debug:
                                nc.sync.dma_start(
                                    out=dbg[dbg_key][:].rearrange(
                                        "r t c -> (r t) c")[128 * wg:128 * (wg + 1), :],
                                    in_=x_sb[:, wg, :])

                        prev = None
                        for w in range(HALFW):
                            pS, em = t_fa(w)
                            if prev is not None:
                                t_back(w - 1, prev)
                            prev = t_fb(w, pS, em)
                        t_back(HALFW - 1, prev)

        temporal("t1", "t1")

        # =====================================================================
        # Cross-attention
        # =====================================================================
        with ExitStack() as ph:
            wp = ph.enter_context(tc.tile_pool(name="wpX", bufs=1))
            zp = ph.enter_context(tc.tile_pool(name="zpX", bufs=2))
            qp = ph.enter_context(tc.tile_pool(name="qpX", bufs=2))

            wqx = load_w_cin(wp, "a2_wq", C)
            wkc = load_w_cin(wp, "a2_wk", CTXD)
            wvc = load_w_cin(wp, "a2_wv", CTXD)
            wox = load_wo(wp, "a2_wo")
            ctx_sb = wp.tile([128, CTXCH, 77], BF16, tag="ctx")
            nc.sync.dma_start(out=ctx_sb[:],
                              in_=ctxT_in[:].rearrange("(a p) m -> p a m", p=128))

            with ExitStack() as hs:
                psk = hs.enter_context(tc.tile_pool(name="pskX", bufs=2,
                                                    space="PSUM"))
                kctxT = wp.tile([DH, HEADS, 77], BF16, tag="kctx")
                pk = psk.tile([DH, HEADS, 128], F32, tag="pk")
                for h in range(HEADS):
                    for ci in range(CTXCH):
                        nc.tensor.matmul(pk[:, h, 0:77],
                                         wkc[:, ci, DH * h:DH * (h + 1)],
                                         ctx_sb[:, ci, :],
                                         start=(ci == 0), stop=(ci == CTXCH - 1))
                pka = pk[:, :, :]
                src = bass.AP(tensor=pk.tensor, offset=pka.offset,
                              ap=[list(pka.ap[0]), [128, HEADS], [1, 77]])
                evict(kctxT[:, :, :], src, w=(1, 1, 1))
                v1x = wp.tile([77, HEADS, DH + 1], BF16, tag="vctx")
                nc.gpsimd.memset(v1x[:], 1.0)
                pv = psk.tile([77, 1024], F32, tag="pvx")
                for o, n in ((0, 512), (512, 128)):
                    for ci in range(CTXCH):
                        nc.tensor.matmul(pv[:, o:o + n], ctx_sb[:, ci, :],
                                         wvc[:, ci, o:o + n],
                                         start=(ci == 0), stop=(ci == CTXCH - 1))
                v1a = v1x[:, :, :]
                dst = bass.AP(tensor=v1x.tensor, offset=v1a.offset,
                              ap=[list(v1a.ap[0]), [DH + 1, HEADS], [1, DH]])
                evict(dst, pv[:, 0:C], w=(1, 1, 1))

            for half in range(2):
                wlo = half * HALFW
                zT = zp.tile([128, CHUNKS, HTOK], BF16, tag="zTx")
                qT = qp.tile([DH, HEADS, HTOK], BF16, tag="qx")
                with ExitStack() as hs:
                    psz = hs.enter_context(tc.tile_pool(name="pszX", bufs=2,
                                                        space="PSUM"))
                    ln_fm(psz, zp, lambda w: x_sb[:, wlo + w, :], zT, HALFW)
                    proj_fm(psz, zT, wqx, qT, HTOK)
                with ExitStack() as hs:
                    pss = hs.enter_context(tc.tile_pool(name="pssX", bufs=2,
                                                        space="PSUM"))
                    psB = hs.enter_context(tc.tile_pool(name="psBX", bufs=2,
                                                        space="PSUM"))
                    eS = qp.tile([77, HEADS, HTOK], BF16, tag="eSx")
                    for h in range(HEADS):
                        for o in range(0, HTOK, 512):
                            n = min(512, HTOK - o)
                            ps = pss.tile([77, 512], F32, tag="psx")
                            nc.tensor.matmul(ps[:, 0:n], kctxT[:, h, :],
                                             qT[:, h, o:o + n],
                                             start=True, stop=True)
                            nc.scalar.activation(out=eS[:, h, o:o + n],
                                                 in_=ps[:, 0:n],
                                                 func=AF.Exp, scale=SCALE)
                    def x_fa(w):
                        oX = psB.tile([128, 1024], F32, tag="oX")
                        for h in range(HEADS):
                            nc.tensor.matmul(oX[:, 128 * h:128 * h + DH + 1],
                                             eS[:, h, 128 * w:128 * (w + 1)],
                                             v1x[:, h, :],
                                             start=True, stop=True)
                        return oX

                    def x_fb(w, oX):
                        rz = small.tile([128, HEADS], F32, tag="rzx")
                        oXa = oX[:, :]
                        zv = bass.AP(tensor=oX.tensor, offset=oXa.offset + DH,
                                     ap=[list(oXa.ap[0]), [128, HEADS]])
                        nc.vector.reciprocal(out=rz[:], in_=zv)
                        oN = zp.tile([128, HEADS, DH], BF16, tag="oNx")
                        src = bass.AP(tensor=oX.tensor, offset=oXa.offset,
                                      ap=[list(oXa.ap[0]), [128, HEADS],
                                          [1, DH]])
                        rza = rz[:, :]
                        rzb = bass.AP(tensor=rz.tensor, offset=rza.offset,
                                      ap=[list(rza.ap[0]), [1, HEADS], [0, DH]])
                        nc.vector.tensor_tensor(out=oN[:], in0=src, in1=rzb,
                                                op=ALU.mult)
                        pt = psB.tile([DH, HEADS, 128], BF16, tag="ptx")
                        for h in range(HEADS):
                            nc.tensor.transpose(pt[:, h, :], oN[:, h, :],
                                                identb[:])
                        oTx = zp.tile([DH, HEADS, 128], BF16, tag="oTx")
                        evict(oTx[:], pt[:], w=(2, 1, 0))
                        return oTx

                    def x_back(w, oTx):
                        wg = wlo + w
                        wo_resid(psB, "oX", oTx, 0, 128, wox, x_sb[:, wg, :])
                        if debug:
                            nc.sync.dma_start(
                                out=dbg["x2"][:].rearrange(
                                    "r t c -> (r t) c")[128 * wg:128 * (wg + 1), :],
                                in_=x_sb[:, wg, :])

                    prev = None
                    for w in range(HALFW):
                        oX = x_fa(w)
                        if prev is not None:
                            x_back(w - 1, prev)
                        prev = x_fb(w, oX)
                    x_back(HALFW - 1, prev)

        temporal("t2", "t2")

        # =====================================================================
        # GEGLU FFN per window. ff_w1 cols host-permuted into rounds of
        # (4 a-chunks, 4 gate-chunks); a-chunk order preserved for ff_w2.
        # =====================================================================
        with ExitStack() as ph:
            wp = ph.enter_context(tc.tile_pool(name="wpF", bufs=1))
            zp = ph.enter_context(tc.tile_pool(name="zpF", bufs=2))
            hp = ph.enter_context(tc.tile_pool(name="hpF", bufs=2))
            psp = ph.enter_context(tc.tile_pool(name="psF", bufs=2, space="PSUM"))
            psx = ph.enter_context(tc.tile_pool(name="psxF", bufs=1, space="PSUM"))
            psh = ph.enter_context(tc.tile_pool(name="pshF", bufs=2, space="PSUM"))

            w1 = wp.tile([128, CHUNKS, 2 * FFI], BF16, tag="w1")
            nc.sync.dma_start(out=w1[:],
                              in_=wts["ff_w1"][:].rearrange("(a p) n -> p a n",
                                                            p=128))
            w2 = wp.tile([128, NG2, C], BF16, tag="w2")
            nc.sync.dma_start(out=w2[:],
                              in_=wts["ff_w2"][:].rearrange("(a p) n -> p a n",
                                                            p=128))

            for w in range(NWIN):
                zT = zp.tile([128, CHUNKS, 128], BF16, tag="zTf")
                ln_fm(psp, zp, lambda _: x_sb[:, w, :], zT, 1)
                uT = hp.tile([128, NG2, 128], BF16, tag="uT")
                for r in range(5):
                    ph_ = psh.tile([128, 8, 128], F32, tag="ph")
                    for co in range(8):
                        gcol = 1024 * r + 128 * co
                        for ci in range(CHUNKS):
                            nc.tensor.matmul(ph_[:, co, :],
                                             w1[:, ci, gcol:gcol + 128],
                                             zT[:, ci, :],
                                             start=(ci == 0), stop=(ci == CHUNKS - 1))
                    gl = hp.tile([128, 4, 128], BF16, tag="gelu")
                    nc.scalar.activation(out=gl[:], in_=ph_[:, 4:8, :],
                                         func=AF.Gelu)
                    nc.vector.tensor_tensor(out=uT[:, 4 * r:4 * r + 4, :],
                                            in0=ph_[:, 0:4, :], in1=gl[:],
                                            op=ALU.mult)
                px = psx.tile([128, 1024], F32, tag="px")
                for o, n in ((0, 512), (512, 128)):
                    for ci in range(NG2):
                        nc.tensor.matmul(px[:, o:o + n], uT[:, ci, :],
                                         w2[:, ci, o:o + n],
                                         start=(ci == 0), stop=(ci == NG2 - 1))
                nc.vector.scalar_tensor_tensor(
                    out=x_sb[:, w, :], in0=px[:, 0:C], scalar=1.0,
                    in1=x_sb[:, w, :], op0=ALU.mult, op1=ALU.add)
                nc.sync.dma_start(
                    out=out_final[:].rearrange(
                        "r t c -> (r t) c")[128 * w:128 * (w + 1), :],
                    in_=x_sb[:, w, :])

    if not nc.is_finalized():
        nc.finalize()
    return nc


# ----------------------------------------------------------------------------
# host side
# ----------------------------------------------------------------------------

def _bf(a):
    return np.asarray(a, dtype=ml_dtypes.bfloat16)


def prepare_inputs(inputs):
    f = {k: np.asarray(v, dtype=np.float32) for k, v in inputs.items()}
    shared = {}

    def fold(g, b, wname):
        wf = f[wname]
        bias = f[b] @ wf
        assert np.abs(bias).max() < 1e-6, f"nonzero folded bias for {wname}"
        return f[g][:, None] * wf

    for k in ("a1_bo", "a2_bo", "t1_bo", "t2_bo", "ff_b1", "ff_b2"):
        assert np.abs(f[k]).max() < 1e-6, f"nonzero bias {k} unsupported"

    for p, gk, bk_ in (("a1", "g1", "b1"), ("t1", "g4", "b4"),
                       ("t2", "g5", "b5")):
        for kind in ("wq", "wk", "wv"):
            shared[f"{p}_{kind}"] = _bf(fold(gk, bk_, f"{p}_{kind}"))
    shared["a2_wq"] = _bf(fold("g2", "b2", "a2_wq"))
    shared["a2_wk"] = _bf(f["a2_wk"])
    shared["a2_wv"] = _bf(f["a2_wv"])
    for p in ("a1", "a2", "t1", "t2"):
        shared[f"{p}_wo"] = _bf(
            f[f"{p}_wo"].reshape(HEADS, DH, C).transpose(1, 0, 2))
    for p in ("t1", "t2"):
        shared[f"{p}_rkT"] = _bf(f[f"{p}_rk"].T)
        rv = f[f"{p}_rv"]
        rvs = np.zeros((16, T, DH), np.float32)
        for t in range(T):
            for j in range(16):
                rvs[j, t] = rv[j - t + MAXREL]
        shared[f"{p}_rvs"] = _bf(rvs)
    w1f = fold("g3", "b3", "ff_w1")
    a_, g_ = w1f[:, :FFI], w1f[:, FFI:]
    cols = []
    for r in range(5):
        cols.append(a_[:, 512 * r:512 * (r + 1)])
        cols.append(g_[:, 512 * r:512 * (r + 1)])
    shared["ff_w1"] = _bf(np.concatenate(cols, axis=1))
    shared["ff_w2"] = _bf(f["ff_w2"])
    m = np.zeros((128, 128), np.float32)
    for g in range(8):
        m[16 * g:16 * (g + 1), 16 * g:16 * (g + 1)] = 1.0
    shared["bd_mask"] = _bf(m)

    x = f["x"]
    ctx = f["context"]
    in_maps = []
    for core in range(NCORES):
        im = dict(shared)
        xs = np.empty((NG, SEQ, C), np.float32)
        for g in range(NG):
            bt = core + 8 * g
            b, t = bt // T, bt % T
            xs[g] = x[b, :, t].reshape(C, SEQ).T
        im["xs_in"] = xs
        im["ctxT"] = _bf(ctx[core // 4].T.copy())
        in_maps.append(im)
    return in_maps


_PROGRAM_CACHE = {}


def run(inputs, debug=False, trace=False):
    key = "dbg" if debug else "plain"
    if key not in _PROGRAM_CACHE:
        _PROGRAM_CACHE[key] = build_program(debug=debug)
    nc = _PROGRAM_CACHE[key]
    in_maps = prepare_inputs(inputs)
    from concourse.bass_utils import run_bass_kernel_spmd
    res = run_bass_kernel_spmd(nc, in_maps, list(range(NCORES)), trace=trace)
    outs = res.results
    full = np.empty((B * H * W, T, C), np.float32)
    for core in range(NCORES):
        full[NR * core:NR * (core + 1)] = outs[core]["out"]
    y = full.reshape(B, H, W, T, C).transpose(0, 4, 3, 1, 2)
    return y, res, outs


def kernel(**inputs):
    y, _, _ = run(inputs)
    return y.astype(np.float32)
